# revision 24
# baseline (speedup 1.0000x reference)
"""
MLA attention (DeepSeek-style) on 8 TRN2 NeuronCores.

Sharding:
  phase 1 (LoRA-A projection + RMSNorm): sharded over sequence (256 rows/core),
    result transposed to feature-major and AllGathered (bf16 latents).
    The kv+rope latent columns are computed first (kt-streaming loop with
    4-ktile batched weight DMAs) and gathered in an early collective that
    overlaps the q-latent loop. A tiny warm-up AllGather issued at kernel
    start absorbs the runtime's first-collective barrier (~50us) so the kv
    gather starts at its trigger. The q latents are gathered in two 768-col
    chunks so the q up-projection can start on chunk 0 while chunk 1 is in
    flight. Latent stores ride the scalar HWDGE ring so the q weight stream
    (sync ring) never queues behind them. The RMS 1/rms is folded into the
    latent transposes as a diagonal stationary matrix.
  phase 2 (q/kv up-proj, attention, o_proj): sharded over heads (4 heads/core),
    w_o input-dim sharded; partial outputs (bf16) summed on the host (the
    all-reduce). The kv up-projection (k_nope/V) is interleaved with the
    chunked q up-projection to keep the PE busy during the q gather.

All heavy matmuls run in bf16 with fp32 PSUM accumulation.
Everything feature-major ("X^T" layout [feature, seq]) in phase 2 so no big
transposes are needed:
  scores^T[sk, sq] accumulated from k^T/q^T; softmax normalizer via ones-matmul;
  (A @ V)^T = matmul(lhsT=V_rowmajor, rhs=A^T); o_proj consumes (A@V)^T directly.
The rope halves of the q up-projection are packed in head pairs (one 128-row
stationary per pair); the scores' rope matmuls then read qTB_pair / a
duplicated k_pe tile at partition offset 64 for odd heads (PE tile_position).
Attention is software-pipelined: AV matmuls trail their scores by two tiles,
each pair's rowsum matmuls run as one same-bank batch (single ones-LDWEIGHTS),
1/rowsum is exp(-ln(x)) on ACT, and the renorm broadcast is a bf16 K=1 matmul
deferred by one (head, sq-block) pair.
The causal diagonal masks are generated on-device (gpsimd affine_select).
"""

import os
import sys
from contextlib import ExitStack

import numpy as np

for _p in ("/opt/trn_rl_repo", "/root/.axon_site/_ro/trn_rl_repo"):
    if os.path.isdir(_p) and _p not in sys.path:
        sys.path.insert(0, _p)

import ml_dtypes  # noqa: E402

import concourse.bacc as bacc  # noqa: E402
import concourse.bass as bass  # noqa: E402
import concourse.mybir as mybir  # noqa: E402
import concourse.tile as tile  # noqa: E402
from concourse import bass_isa  # noqa: E402
from concourse.bass_utils import run_bass_kernel_spmd  # noqa: E402
from concourse.masks import make_identity  # noqa: E402

# ---------------------------------------------------------------- constants
NCORES = 8
S = 2048
SL = S // NCORES  # 256 local rows in phase 1
HID = 4096
Q_LORA = 1536
KV_LORA = 512
ROPE = 64
C = Q_LORA + KV_LORA + ROPE  # 2112
CKV_R = KV_LORA + ROPE  # 576 kv+rope latent cols
NOPE = 128
V_DIM = 128
H = 32
HL = H // NCORES  # 4 local heads
NPAIR = HL // 2  # 2 head pairs (rope packing)
Q_HEAD = NOPE + ROPE  # 192
EPS = 1e-6
NEG = -1e9

F32 = mybir.dt.float32
BF16 = mybir.dt.bfloat16
U8 = mybir.dt.uint8

CQ_TILES = Q_LORA // 128  # 12
CQ_HALF = CQ_TILES // 2  # 6 (gather chunk size)
CKV_TILES = KV_LORA // 128  # 4
HT_TILES = HID // 128  # 32
S_TILES = S // 128  # 16
SQB = 512
NSQB = S // SQB  # 4
EB = 512
NEB = HID // EB  # 8


# ---------------------------------------------------------------- program
def build_program() -> bass.Bass:
    nc = bacc.Bacc(
        "TRN2",
        target_bir_lowering=False,
        debug=False,
        num_devices=NCORES,
    )

    hid_d = nc.declare_dram_parameter("hid", [SL, HID], BF16, isOutput=False)
    wa_d = nc.declare_dram_parameter("wa", [HID, C], BF16, isOutput=False)
    wqb_d = nc.declare_dram_parameter("wqb", [Q_LORA, HL * Q_HEAD], BF16, isOutput=False)
    wkvb_d = nc.declare_dram_parameter(
        "wkvb", [KV_LORA, HL * (NOPE + V_DIM)], BF16, isOutput=False
    )
    wo_d = nc.declare_dram_parameter("wo", [HL * V_DIM, HID], BF16, isOutput=False)
    out_d = nc.declare_dram_parameter("out", [S, HID], BF16, isOutput=True)

    # collective bounce buffers (internal DRAM)
    cc_in_kv = nc.dram_tensor("cc_in_kv", [CKV_R, SL], BF16)
    cc_out_kv = nc.dram_tensor("cc_out_kv", [NCORES, CKV_R, SL], BF16, addr_space="Shared")
    cc_in_q0 = nc.dram_tensor("cc_in_q0", [CQ_HALF * 128, SL], BF16)
    cc_out_q0 = nc.dram_tensor(
        "cc_out_q0", [NCORES, CQ_HALF * 128, SL], BF16, addr_space="Shared"
    )
    cc_in_q1 = nc.dram_tensor("cc_in_q1", [CQ_HALF * 128, SL], BF16)
    cc_out_q1 = nc.dram_tensor(
        "cc_out_q1", [NCORES, CQ_HALF * 128, SL], BF16, addr_space="Shared"
    )

    with tile.TileContext(nc, num_cores=NCORES) as tc, ExitStack() as stack:
        # ---------------- small persistent constants (gpsimd, pre-warmup)
        misc = stack.enter_context(tc.tile_pool(name="misc", bufs=1))
        ident = misc.tile([128, 128], BF16, tag="ident", name="ident")
        make_identity(nc, ident[:])
        ones_sb = misc.tile([128, 1], BF16, tag="ones", name="ones")
        nc.gpsimd.memset(ones_sb[:], 1.0)
        mask_sb = misc.tile([128, 4 * SQB], F32, tag="mask", name="mask")
        eps_sb = misc.tile([128, 1], F32, tag="eps", name="eps")
        nc.gpsimd.memset(eps_sb[:], EPS)
        onesr_sb = misc.tile([1, 128], BF16, tag="onesr", name="onesr")
        nc.gpsimd.memset(onesr_sb[:], 1.0)
        # causal diagonal masks generated on-device, before the collectives
        # claim the gpsimd queue (a collective trigger blocks its engine until
        # the collective completes)
        nc.gpsimd.memset(mask_sb[:], 0.0)
        for dd in range(4):
            nc.gpsimd.affine_select(
                out=mask_sb[:, dd * SQB : (dd + 1) * SQB],
                in_=mask_sb[:, dd * SQB : (dd + 1) * SQB],
                compare_op=mybir.AluOpType.is_ge,
                fill=NEG,
                base=-128 * dd,
                # keep (0) where f - p - 128*d >= 0 else NEG
                pattern=[[1, SQB]],
                channel_multiplier=-1,
            )

        # PE warm-up burst: ~5us of dummy matmuls while the first DMAs land,
        # so the HAM activity monitor lifts the 4/8 (half-clock) gate before
        # the real phase-1 matmuls start (saves their cold-clock penalty)
        with ExitStack() as wu:
            wu_pool = wu.enter_context(tc.tile_pool(name="wup", bufs=2, space="PSUM"))
            wu_sink = misc.tile([1, 128], F32, tag="wusink", name="wusink")
            wps = None
            for _ in range(64):
                wps = wu_pool.tile([128, 128], F32, tag="wup", name="wps")
                nc.tensor.matmul(wps[:], ident[:], ident[:], start=True, stop=True)
            nc.vector.tensor_copy(wu_sink[:], wps[0:1, :])

        # phase-2 weights + kv latents: allocated up front; DMAs issued at the
        # point in each engine's stream where their data is needed next
        wkvb_pool = stack.enter_context(tc.tile_pool(name="wkvb", bufs=1))
        wkvb_sb = [
            wkvb_pool.tile(
                [128, HL * (NOPE + V_DIM)], BF16, tag=f"wkvb{kt}", name=f"wkvb{kt}"
            )
            for kt in range(CKV_TILES)
        ]
        wqb_pool = stack.enter_context(tc.tile_pool(name="wqb", bufs=1))
        wqb_sb = [
            wqb_pool.tile([128, HL * Q_HEAD], BF16, tag=f"wqb{kt}", name=f"wqb{kt}")
            for kt in range(CQ_TILES)
        ]
        latkv = stack.enter_context(tc.tile_pool(name="latkv", bufs=1))
        latkv_sb = [
            latkv.tile([128, S], BF16, tag=f"latkv{i}", name=f"latkv{i}")
            for i in range(CKV_TILES)
        ]
        # k_pe duplicated into both partition halves so odd heads can read it
        # (and their rope-q) at partition offset 64
        kpe2 = latkv.tile([128, S], BF16, tag="kpe2", name="kpe2")

        # ---------------- phase 1: a-projection on local rows (kt-streaming)
        with ExitStack() as p1:
            p1_pool = p1.enter_context(tc.tile_pool(name="p1", bufs=1))
            hid_sb = [
                p1_pool.tile([128, HID], BF16, tag=f"hid{s2}", name=f"hid{s2}")
                for s2 in range(2)
            ]
            # hid on the scalar ring so the wa weight stream (sync ring)
            # starts at t=0 in parallel
            for half in range(2):
                for s2 in range(2):
                    nc.scalar.dma_start(
                        hid_sb[s2][:, half * 2048 : (half + 1) * 2048],
                        hid_d[s2 * 128 : (s2 + 1) * 128, half * 2048 : (half + 1) * 2048],
                    )
            hidT = [
                p1_pool.tile([128, SL], BF16, tag=f"hidT{ht}", name=f"hidT{ht}")
                for ht in range(HT_TILES)
            ]
            lat_sb = [
                p1_pool.tile([128, C], BF16, tag=f"lat{s2}", name=f"lat{s2}")
                for s2 in range(2)
            ]
            stat = p1_pool.tile([128, 24], F32, tag="stat", name="stat")
            rms_scratch = p1_pool.tile([128, 512], F32, tag="rmssc", name="rmssc")
            diag_sb = [
                p1_pool.tile([128, 128], BF16, tag=f"diag{s2}", name=f"diag{s2}")
                for s2 in range(2)
            ]
            # local latents^T staging (feature-major, [*, SL])
            latTq_all = p1_pool.tile(
                [128, CQ_TILES * SL], BF16, tag="latTqa", name="latTqa"
            )
            latTkv_all = p1_pool.tile(
                [128, CKV_TILES * SL], BF16, tag="latTkva", name="latTkva"
            )
            latTkv_rope = p1_pool.tile([ROPE, SL], BF16, tag="latTkvr", name="latTkvr")

            kvw_pool = p1.enter_context(tc.tile_pool(name="kvw", bufs=8))
            qw_pool = p1.enter_context(tc.tile_pool(name="qw", bufs=8))

            # ---- kv + rope latents first (two 288-col psum accumulators x 2 s2)
            KVG = 4  # kt per kv weight DMA
            wa_kv_view = wa_d[:, Q_LORA : Q_LORA + CKV_R].rearrange(
                "(k p) c -> p k c", p=128
            )
            with ExitStack() as sA:
                tps_pool = sA.enter_context(
                    tc.tile_pool(name="tpsA", bufs=4, space="PSUM")
                )
                acc_pool = sA.enter_context(
                    tc.tile_pool(name="accA", bufs=1, space="PSUM")
                )

                def transpose_hid(ht):
                    for s2 in range(2):
                        pt = tps_pool.tile([128, 128], BF16, tag="tps", name="tps")
                        nc.tensor.transpose(
                            pt[:], hid_sb[s2][:, ht * 128 : (ht + 1) * 128], ident[:]
                        )
                        nc.vector.tensor_copy(
                            hidT[ht][:, s2 * 128 : (s2 + 1) * 128], pt[:]
                        )

                kv_acc = [
                    [
                        acc_pool.tile(
                            [128, 288], F32, tag=f"kvacc{s2}{b}", name=f"kvacc{s2}{b}"
                        )
                        for b in range(2)
                    ]
                    for s2 in range(2)
                ]
                transpose_hid(0)
                transpose_hid(1)
                for g in range(HT_TILES // KVG):
                    t = kvw_pool.tile(
                        [128, KVG * CKV_R], BF16, tag="kvw", name=f"kvw{g}"
                    )
                    nc.sync.dma_start(
                        t[:].rearrange("p (k c) -> p k c", k=KVG),
                        wa_kv_view[:, g * KVG : (g + 1) * KVG],
                    )
                    for j in range(KVG):
                        kt = g * KVG + j
                        if kt + 2 < HT_TILES:
                            transpose_hid(kt + 2)
                        for s2 in range(2):
                            for b in range(2):
                                nc.tensor.matmul(
                                    kv_acc[s2][b][:],
                                    hidT[kt][:, s2 * 128 : (s2 + 1) * 128],
                                    t[:, j * CKV_R + b * 288 : j * CKV_R + (b + 1) * 288],
                                    start=(kt == 0),
                                    stop=(kt == HT_TILES - 1),
                                )
                for s2 in range(2):
                    for b in range(2):
                        nc.scalar.copy(
                            lat_sb[s2][:, Q_LORA + b * 288 : Q_LORA + (b + 1) * 288],
                            kv_acc[s2][b][:],
                        )

            # wkvb prefetch on the scalar ring (executes ~kv-copy time, well
            # before the kv up-projection needs it)
            for kt in range(CKV_TILES):
                nc.scalar.dma_start(
                    wkvb_sb[kt][:], wkvb_d[kt * 128 : (kt + 1) * 128, :]
                )

            def rms_diag(col0, ncols, stat_base):
                """1/rms of lat_sb[:, col0:col0+ncols] per row, folded into a
                per-s2 diagonal matrix applied by the latent transposes."""
                nch = (ncols + 511) // 512
                for s2 in range(2):
                    for ch in range(nch):
                        w = min(512, ncols - ch * 512)
                        src = lat_sb[s2][:, col0 + ch * 512 : col0 + ch * 512 + w]
                        nc.vector.scalar_tensor_tensor(
                            rms_scratch[:, 0:w],
                            src,
                            1.0,
                            src,
                            op0=mybir.AluOpType.mult,
                            op1=mybir.AluOpType.mult,
                            accum_out=stat[:, stat_base + 3 * s2 + ch
                                           : stat_base + 3 * s2 + ch + 1],
                        )
                    for ch in range(1, nch):
                        nc.vector.tensor_add(
                            stat[:, stat_base + 3 * s2 : stat_base + 3 * s2 + 1],
                            stat[:, stat_base + 3 * s2 : stat_base + 3 * s2 + 1],
                            stat[:, stat_base + 3 * s2 + ch
                                 : stat_base + 3 * s2 + ch + 1],
                        )
                for s2 in range(2):
                    nc.scalar.activation(
                        stat[:, stat_base + 6 + s2 : stat_base + 7 + s2],
                        stat[:, stat_base + 3 * s2 : stat_base + 3 * s2 + 1],
                        mybir.ActivationFunctionType.Sqrt,
                        scale=1.0 / ncols,
                        bias=eps_sb[:],
                    )
                for s2 in range(2):
                    nc.vector.reciprocal(
                        stat[:, stat_base + 8 + s2 : stat_base + 9 + s2],
                        stat[:, stat_base + 6 + s2 : stat_base + 7 + s2],
                    )
                for s2 in range(2):
                    nc.vector.tensor_scalar_mul(
                        diag_sb[s2][:],
                        ident[:],
                        stat[:, stat_base + 8 + s2 : stat_base + 9 + s2],
                    )

            def transpose_lat(src_col, w, dst, tps_pool, scaled):
                """dst[:, s2*128...] = (lat_sb[s2][:, src_col:src_col+w])T,
                optionally scaled per seq row (lat.T @ diag(1/rms))."""
                for s2 in range(2):
                    pt = tps_pool.tile([128, 128], F32, tag="tps", name="tpsl")
                    tmat = diag_sb[s2] if scaled else ident
                    nc.tensor.matmul(
                        pt[:w, :],
                        lat_sb[s2][:, src_col : src_col + w],
                        tmat[:],
                        start=True,
                        stop=True,
                    )
                    nc.vector.tensor_copy(
                        dst[:, s2 * 128 : (s2 + 1) * 128], pt[:w, :]
                    )

            with ExitStack() as sB:
                tpsB = sB.enter_context(tc.tile_pool(name="tpsB", bufs=2, space="PSUM"))
                rms_diag(Q_LORA, KV_LORA, 0)
                for ct in range(CKV_TILES):
                    transpose_lat(
                        Q_LORA + ct * 128,
                        128,
                        latTkv_all[:, ct * SL : (ct + 1) * SL],
                        tpsB,
                        scaled=True,
                    )
                transpose_lat(Q_LORA + KV_LORA, ROPE, latTkv_rope[:], tpsB, scaled=False)
                # latent stores on the scalar ring: the q weight stream on the
                # sync ring must not queue behind them
                nc.scalar.dma_start(
                    cc_in_kv[0:KV_LORA].rearrange("(ct p) s -> p ct s", p=128),
                    latTkv_all[:].rearrange("p (ct s) -> p ct s", ct=CKV_TILES),
                )
                nc.scalar.dma_start(
                    cc_in_kv[KV_LORA : KV_LORA + ROPE, :], latTkv_rope[:]
                )
            nc.gpsimd.collective_compute(
                "AllGather",
                mybir.AluOpType.bypass,
                replica_groups=[list(range(NCORES))],
                ins=[cc_in_kv[:].opt()],
                outs=[cc_out_kv[:].opt()],
            )

            # ---- q latents (three 512-col psum accumulators x 2 s2)
            with ExitStack() as sC:
                accC = sC.enter_context(tc.tile_pool(name="accC", bufs=1, space="PSUM"))
                q_acc = [
                    [
                        accC.tile(
                            [128, 512], F32, tag=f"qacc{s2}{b}", name=f"qacc{s2}{b}"
                        )
                        for b in range(3)
                    ]
                    for s2 in range(2)
                ]
                QG = 2  # kt per q weight DMA
                wa_q_view = wa_d[:, 0:Q_LORA].rearrange("(k p) c -> p k c", p=128)
                for g in range(HT_TILES // QG):
                    t = qw_pool.tile(
                        [128, QG * Q_LORA], BF16, tag="qw", name=f"qw{g}"
                    )
                    nc.sync.dma_start(
                        t[:].rearrange("p (k c) -> p k c", k=QG),
                        wa_q_view[:, g * QG : (g + 1) * QG],
                    )
                    for j in range(QG):
                        kt = g * QG + j
                        for s2 in range(2):
                            for b in range(3):
                                nc.tensor.matmul(
                                    q_acc[s2][b][:],
                                    hidT[kt][:, s2 * 128 : (s2 + 1) * 128],
                                    t[:, j * Q_LORA + b * 512 : j * Q_LORA + (b + 1) * 512],
                                    start=(kt == 0),
                                    stop=(kt == HT_TILES - 1),
                                )
                for s2 in range(2):
                    for b in range(3):
                        nc.scalar.copy(
                            lat_sb[s2][:, b * 512 : (b + 1) * 512], q_acc[s2][b][:]
                        )

            # wqb prefetch on the sync ring right behind the q weight stream
            for kt in range(CQ_TILES):
                nc.sync.dma_start(wqb_sb[kt][:], wqb_d[kt * 128 : (kt + 1) * 128, :])

            with ExitStack() as sD:
                tpsD = sD.enter_context(tc.tile_pool(name="tpsD", bufs=2, space="PSUM"))
                rms_diag(0, Q_LORA, 12)
                # chunk 0: ct 0-5 -> store -> gather; chunk 1: ct 6-11
                for ct in range(CQ_HALF):
                    transpose_lat(
                        ct * 128,
                        128,
                        latTq_all[:, ct * SL : (ct + 1) * SL],
                        tpsD,
                        scaled=True,
                    )
                nc.scalar.dma_start(
                    cc_in_q0[:].rearrange("(ct p) s -> p ct s", p=128),
                    latTq_all[:, 0 : CQ_HALF * SL].rearrange(
                        "p (ct s) -> p ct s", ct=CQ_HALF
                    ),
                )
                for ct in range(CQ_HALF, CQ_TILES):
                    transpose_lat(
                        ct * 128,
                        128,
                        latTq_all[:, ct * SL : (ct + 1) * SL],
                        tpsD,
                        scaled=True,
                    )
                nc.scalar.dma_start(
                    cc_in_q1[:].rearrange("(ct p) s -> p ct s", p=128),
                    latTq_all[:, CQ_HALF * SL :].rearrange(
                        "p (ct s) -> p ct s", ct=CQ_HALF
                    ),
                )
            nc.gpsimd.collective_compute(
                "AllGather",
                mybir.AluOpType.bypass,
                replica_groups=[list(range(NCORES))],
                ins=[cc_in_q0[:].opt()],
                outs=[cc_out_q0[:].opt()],
            )
            nc.gpsimd.collective_compute(
                "AllGather",
                mybir.AluOpType.bypass,
                replica_groups=[list(range(NCORES))],
                ins=[cc_in_q1[:].opt()],
                outs=[cc_out_q1[:].opt()],
            )

            # gathered kv latents into SBUF. The sync/scalar rings are idle
            # once phase 1 drains and (unlike gpsimd) not blocked by the
            # in-flight q collectives; each entry's gate time is later than
            # the previous entry's, so no head-of-line blocking.
            cc_kv_view = cc_out_kv[:].rearrange("j c s -> c j s")
            for i in range(CKV_TILES):
                eng = nc.sync if i < 2 else nc.scalar
                eng.dma_start(
                    latkv_sb[i][:].rearrange("c (j s) -> c j s", j=NCORES),
                    cc_kv_view[i * 128 : (i + 1) * 128],
                )
            for half in range(2):
                nc.scalar.dma_start(
                    kpe2[half * 64 : (half + 1) * 64, :].rearrange(
                        "c (j s) -> c j s", j=NCORES
                    ),
                    cc_kv_view[KV_LORA : KV_LORA + ROPE],
                )

        # ---------------- phase 2
        kvpool = stack.enter_context(tc.tile_pool(name="kvpool", bufs=1))
        knopeT = [
            kvpool.tile([128, S], BF16, tag=f"knopeT{h}", name=f"knopeT{h}")
            for h in range(HL)
        ]
        v_sb = [
            kvpool.tile([128, HL * V_DIM], BF16, tag=f"v{st}", name=f"v{st}")
            for st in range(S_TILES)
        ]
        qT = stack.enter_context(tc.tile_pool(name="qT", bufs=1))
        qTA = [qT.tile([128, S], BF16, tag=f"qTA{h}", name=f"qTA{h}") for h in range(HL)]
        # rope q of head pair (2p, 2p+1) stacked in partition halves
        qTB = [qT.tile([128, S], BF16, tag=f"qTB{p}", name=f"qTB{p}") for p in range(NPAIR)]
        outT_pool = stack.enter_context(tc.tile_pool(name="outT", bufs=1))
        outT = [
            outT_pool.tile([128, S], BF16, tag=f"outT{h}", name=f"outT{h}")
            for h in range(HL)
        ]

        # kv up-proj interleaved with the chunked q up-proj (one scope so the
        # PSUM pools coexist: pkv 2 banks + pq 6 banks = 8)
        with ExitStack() as p2q:
            latq = p2q.enter_context(tc.tile_pool(name="latq", bufs=1))
            latq_sb = [
                latq.tile([128, S], BF16, tag=f"latq{ct}", name=f"latq{ct}")
                for ct in range(CQ_TILES)
            ]
            cc_q_views = [
                cc_out_q0[:].rearrange("j c s -> c j s"),
                cc_out_q1[:].rearrange("j c s -> c j s"),
            ]
            # gpsimd is blocked until the q collectives complete, so only the
            # sync/scalar rings carry the gathered-latent loads
            for ct in range(CQ_TILES):
                half, cth = divmod(ct, CQ_HALF)
                eng = (nc.sync, nc.scalar)[ct % 2]
                eng.dma_start(
                    latq_sb[ct][:].rearrange("c (j s) -> c j s", j=NCORES),
                    cc_q_views[half][cth * 128 : (cth + 1) * 128],
                )
            pkv_pool = p2q.enter_context(tc.tile_pool(name="pkv", bufs=2, space="PSUM"))
            pq_pool = p2q.enter_context(tc.tile_pool(name="pq", bufs=1, space="PSUM"))
            fill_sink = p2q.enter_context(tc.tile_pool(name="fsink", bufs=1)).tile(
                [1, 128], F32, tag="fsink", name="fsink"
            )

            def knope_upproj():
                for h in range(HL):
                    for skb in range(NSQB):
                        pk = pkv_pool.tile([128, SQB], F32, tag="pkv", name="pk")
                        for kt in range(CKV_TILES):
                            nc.tensor.matmul(
                                pk[:],
                                wkvb_sb[kt][
                                    :, h * (NOPE + V_DIM) : h * (NOPE + V_DIM) + NOPE
                                ],
                                latkv_sb[kt][:, skb * SQB : (skb + 1) * SQB],
                                start=(kt == 0),
                                stop=(kt == CKV_TILES - 1),
                            )
                        nc.vector.tensor_copy(
                            knopeT[h][:, skb * SQB : (skb + 1) * SQB], pk[:]
                        )

            def v_upproj(st_range):
                for st in st_range:
                    pv = pkv_pool.tile([128, HL * V_DIM], F32, tag="pkv", name="pv")
                    for kt in range(CKV_TILES):
                        rhs = wkvb_sb[kt][:].rearrange(
                            "c (h d) -> c h d", h=HL
                        )[:, :, NOPE:]
                        nc.tensor.matmul(
                            pv[:],
                            latkv_sb[kt][:, st * 128 : (st + 1) * 128],
                            rhs,
                            start=(kt == 0),
                            stop=(kt == CKV_TILES - 1),
                        )
                    nc.vector.tensor_copy(v_sb[st][:], pv[:])

            def q_upproj_block(sqb, pqs, kts):
                # wqb cols are host-reordered: [nope h0..h3 | rope h0..h3]
                for kt in kts:
                    for h in range(HL):
                        nc.tensor.matmul(
                            pqs[h][:],
                            wqb_sb[kt][:, h * NOPE : (h + 1) * NOPE],
                            latq_sb[kt][:, sqb * SQB : (sqb + 1) * SQB],
                            start=(kt == 0),
                            stop=(kt == CQ_TILES - 1),
                        )
                    for p in range(NPAIR):
                        nc.tensor.matmul(
                            pqs[HL + p][:],
                            wqb_sb[kt][
                                :, HL * NOPE + p * 128 : HL * NOPE + (p + 1) * 128
                            ],
                            latq_sb[kt][:, sqb * SQB : (sqb + 1) * SQB],
                            start=(kt == 0),
                            stop=(kt == CQ_TILES - 1),
                        )

            def q_copies(sqb, pqs):
                for h in range(HL):
                    nc.scalar.copy(
                        qTA[h][:, sqb * SQB : (sqb + 1) * SQB], pqs[h][:]
                    )
                for p in range(NPAIR):
                    nc.scalar.copy(
                        qTB[p][:, sqb * SQB : (sqb + 1) * SQB], pqs[HL + p][:]
                    )

            def q_psums():
                return [
                    pq_pool.tile([128, SQB], F32, tag=f"pq{u}", name=f"pq{u}")
                    for u in range(HL + NPAIR)
                ]

            def pe_filler(n):
                # dependency-free matmuls keeping the HAM activity window busy
                # across a known gather-wait hole, so the work that follows
                # resumes at full clock instead of the 4/8 cold gate; bounded
                # cost if the gather lands early
                fps = pkv_pool.tile([128, SQB], F32, tag="pkv", name="fill")
                for _ in range(n):
                    nc.tensor.matmul(
                        fps[:, 0:128], ident[:], ident[:], start=True, stop=True
                    )
                nc.vector.tensor_copy(fill_sink[:], fps[0:1, 0:128])

            # emission order tuned to data arrival: kv work (gated on the kv
            # gather) fills the q-gather window; V's second half and PE filler
            # bursts cover the chunk-0 -> chunk-1 gather gaps
            knope_upproj()
            v_upproj(range(0, 8))
            pe_filler(80)
            pqs0 = q_psums()
            q_upproj_block(0, pqs0, range(0, CQ_HALF))
            v_upproj(range(8, S_TILES))
            pe_filler(140)
            q_upproj_block(0, pqs0, range(CQ_HALF, CQ_TILES))
            q_copies(0, pqs0)
            for sqb in range(1, NSQB):
                pqs = q_psums()
                q_upproj_block(sqb, pqs, range(0, CQ_HALF))
                q_upproj_block(sqb, pqs, range(CQ_HALF, CQ_TILES))
                q_copies(sqb, pqs)

        # o_proj weights: loaded late (SBUF freed by the q latents)
        wo_pool = stack.enter_context(tc.tile_pool(name="wo", bufs=1))
        wo_sb = [
            wo_pool.tile([128, HID], BF16, tag=f"wo{h}", name=f"wo{h}")
            for h in range(HL)
        ]
        for h in range(HL):
            nc.sync.dma_start(wo_sb[h][:], wo_d[h * 128 : (h + 1) * 128, :])

        # ---------------- attention (causal, block-skipped) + interleaved o_proj
        # bq-outer so each 512-row sq block's outT completes early; its o_proj
        # block is emitted as soon as the last head's epilogue drains, filling
        # attention-pipeline bubbles and spreading the output DMA.
        # Rowsums: DVE pair-adds halve the exp tiles (bf16), then one
        # accumulating ones-matmul run per (h, bq) over the nk/2 pair-sums
        # (half the PE stream cost of per-tile rowsum matmuls); 1/rowsum is
        # exp(-ln(x)) on ACT and the renorm broadcast a bf16 K=1 matmul,
        # both deferred as in the baseline pipeline.
        with ExitStack() as p2a:
            ps_pool = p2a.enter_context(tc.tile_pool(name="ps", bufs=5, space="PSUM"))
            psum_sum_pool = p2a.enter_context(
                tc.tile_pool(name="psums", bufs=1, space="PSUM")
            )
            psum_o_pool = p2a.enter_context(
                tc.tile_pool(name="psumo", bufs=2, space="PSUM")
            )
            a_pool = p2a.enter_context(tc.tile_pool(name="apool", bufs=10))
            apair_pool = p2a.enter_context(tc.tile_pool(name="apair", bufs=12))
            bc_pool = p2a.enter_context(tc.tile_pool(name="bcpool", bufs=3))

            tile_q = []  # score tiles awaiting their AV matmuls
            ep_q = []  # pairs awaiting the renormalization epilogue

            def drain_tile():
                a, h, bq, tk, nk, po = tile_q.pop(0)
                nc.tensor.matmul(
                    po[:],
                    v_sb[tk][:, h * V_DIM : (h + 1) * V_DIM],
                    a[:],
                    start=(tk == 0),
                    stop=(tk == nk - 1),
                )

            def drain_sums(pair):
                h, bq, po, psum, pair_sums = pair
                for i, ap in enumerate(pair_sums):
                    nc.tensor.matmul(
                        psum[:],
                        ones_sb[:],
                        ap[:],
                        start=(i == 0),
                        stop=(i == len(pair_sums) - 1),
                    )
                # 1/rowsum on DVE (custom op, ~18 bits, rowsum > 0 always):
                # keeps Ln/Exp off the ACT engine, whose activation-table set
                # would thrash against the softmax Exp (1.3us reload per swap)
                rs32 = bc_pool.tile([1, SQB], F32, tag="rs32", name="rs32")
                nc.vector.reciprocal_approx_fast(rs32[:], psum[:])
                rs = bc_pool.tile([1, SQB], BF16, tag="rs", name="rs")
                nc.vector.tensor_copy(rs[:], rs32[:])
                ep_q.append((h, bq, po, rs))

            def drain_epilogue():
                h, bq, po, rs = ep_q.pop(0)
                bc_ps = ps_pool.tile([128, SQB], F32, tag="ps", name="bc_ps")
                nc.tensor.matmul(bc_ps[:], onesr_sb[:], rs[:], start=True, stop=True)
                bc_sb = bc_pool.tile([128, SQB], F32, tag="bc", name="bc_sb")
                nc.scalar.copy(bc_sb[:], bc_ps[:])
                nc.vector.tensor_mul(
                    outT[h][:, bq * SQB : (bq + 1) * SQB], po[:], bc_sb[:]
                )

            prev_pair = None
            for bq in range(NSQB):
                nk = 4 * (bq + 1)
                for h in range(HL):
                    off = 64 * (h % 2)
                    qTBh = qTB[h // 2]
                    po = psum_o_pool.tile([128, SQB], F32, tag="psumo", name="po")
                    psum = psum_sum_pool.tile([1, SQB], F32, tag="psums", name="psum")
                    pair_sums = []
                    pend_a = None
                    for tk in range(nk):
                        ps = ps_pool.tile([128, SQB], F32, tag="ps", name="ps")
                        nc.tensor.matmul(
                            ps[:],
                            knopeT[h][:, tk * 128 : (tk + 1) * 128],
                            qTA[h][:, bq * SQB : (bq + 1) * SQB],
                            start=True,
                            stop=False,
                        )
                        nc.tensor.matmul(
                            ps[:],
                            kpe2[off : off + 64, tk * 128 : (tk + 1) * 128],
                            qTBh[off : off + 64, bq * SQB : (bq + 1) * SQB],
                            start=False,
                            stop=True,
                        )
                        d = tk - 4 * bq
                        if d >= 0:
                            nc.vector.tensor_add(
                                ps[:], ps[:], mask_sb[:, d * SQB : (d + 1) * SQB]
                            )
                        a = a_pool.tile([128, SQB], BF16, tag="a", name="a")
                        nc.scalar.activation(
                            a[:], ps[:], mybir.ActivationFunctionType.Exp
                        )
                        # rowsum pre-reduction: DVE pair-add (bf16)
                        if tk % 2 == 0:
                            pend_a = a
                        else:
                            apair = apair_pool.tile(
                                [128, SQB], BF16, tag="apair", name="apair"
                            )
                            nc.vector.tensor_add(apair[:], pend_a[:], a[:])
                            pair_sums.append(apair)
                        tile_q.append((a, h, bq, tk, nk, po))
                        while len(tile_q) > 3:
                            drain_tile()
                        if tk == 2 and prev_pair is not None:
                            drain_sums(prev_pair)
                            prev_pair = None
                        while len(ep_q) > 1:
                            drain_epilogue()
                    prev_pair = (h, bq, po, psum, pair_sums)
            while tile_q:
                drain_tile()
            if prev_pair is not None:
                drain_sums(prev_pair)
            while ep_q:
                drain_epilogue()

        # ---------------- o_proj (partial: summed across cores on host)
        # kept as a dedicated tail phase: its matmuls have trivially-satisfied
        # dependencies there and pipeline back-to-back on the PE
        with ExitStack() as p2o:
            pe_pool = p2o.enter_context(tc.tile_pool(name="pe", bufs=4, space="PSUM"))
            stage_pool = p2o.enter_context(tc.tile_pool(name="stage", bufs=3))
            for st in range(S_TILES):
                for half in range(2):
                    stg = stage_pool.tile([128, 4 * EB], BF16, tag="stage", name="stg")
                    for ebl in range(4):
                        eb = half * 4 + ebl
                        pe = pe_pool.tile([128, EB], F32, tag="pe", name="pe")
                        for h in range(HL):
                            nc.tensor.matmul(
                                pe[:],
                                outT[h][:, st * 128 : (st + 1) * 128],
                                wo_sb[h][:, eb * EB : (eb + 1) * EB],
                                start=(h == 0),
                                stop=(h == HL - 1),
                            )
                        nc.vector.tensor_copy(
                            stg[:, ebl * EB : (ebl + 1) * EB], pe[:]
                        )
                    nc.gpsimd.dma_start(
                        out_d[
                            st * 128 : (st + 1) * 128,
                            half * 4 * EB : (half + 1) * 4 * EB,
                        ],
                        stg[:],
                    )

    nc.compile()
    return nc


_PROGRAM_CACHE = {}


def _get_program() -> bass.Bass:
    if "nc" not in _PROGRAM_CACHE:
        _PROGRAM_CACHE["nc"] = build_program()
    return _PROGRAM_CACHE["nc"]


def prepare_inputs(
    hidden_states, w_qkv_a, q_a_gamma, w_q_b, kv_a_gamma, w_kv_b, w_o, b_o
):
    """Host-side prep: fold gammas + attention scale into B weights, cast to
    bf16, slice per core."""
    bf = ml_dtypes.bfloat16
    hs = np.asarray(hidden_states, np.float32).reshape(S, HID)
    scale = float(Q_HEAD) ** -0.5
    wqb_eff = (
        np.asarray(w_q_b, np.float32)
        * np.asarray(q_a_gamma, np.float32)[:, None]
        * scale
    )
    wkvb_eff = (
        np.asarray(w_kv_b, np.float32) * np.asarray(kv_a_gamma, np.float32)[:, None]
    )
    wa_bf = np.asarray(w_qkv_a, np.float32).astype(bf)
    hs_bf = hs.astype(bf)

    wqb_r = wqb_eff.reshape(Q_LORA, H, Q_HEAD)
    wkvb_r = wkvb_eff.reshape(KV_LORA, H, NOPE + V_DIM)
    wo_r = np.asarray(w_o, np.float32).reshape(H, V_DIM, HID)

    in_maps = []
    for c in range(NCORES):
        hsl = np.ascontiguousarray(hs_bf[c * SL : (c + 1) * SL])
        wqb_loc = wqb_r[:, c * HL : (c + 1) * HL]  # [Q_LORA, HL, Q_HEAD]
        # column order: [nope h0..h3 | rope h0..h3] (pair-packed rope)
        wqb_c = np.ascontiguousarray(
            np.concatenate(
                [
                    wqb_loc[:, :, :NOPE].reshape(Q_LORA, HL * NOPE),
                    wqb_loc[:, :, NOPE:].reshape(Q_LORA, HL * ROPE),
                ],
                axis=1,
            ).astype(bf)
        )
        wkvb_c = np.ascontiguousarray(
            wkvb_r[:, c * HL : (c + 1) * HL]
            .reshape(KV_LORA, HL * (NOPE + V_DIM))
            .astype(bf)
        )
        wo_c = np.ascontiguousarray(
            wo_r[c * HL : (c + 1) * HL].reshape(HL * V_DIM, HID).astype(bf)
        )
        in_maps.append(
            {
                "hid": hsl,
                "wa": wa_bf,
                "wqb": wqb_c,
                "wkvb": wkvb_c,
                "wo": wo_c,
            }
        )
    return in_maps


def kernel(**inputs) -> np.ndarray:
    in_maps = prepare_inputs(**inputs)
    nc = _get_program()
    res = run_bass_kernel_spmd(nc, in_maps, list(range(NCORES)))
    out = np.zeros((S, HID), np.float64)
    for r in res.results:
        out += np.asarray(r["out"], np.float32)
    out = out.astype(np.float32) + np.asarray(inputs["b_o"], np.float32)[None, :]
    return out.reshape(1, S, HID)


# revision 29
# speedup vs baseline: 1.0136x; 1.0136x over previous
"""
MLA attention (DeepSeek-style) on 8 TRN2 NeuronCores.

Sharding:
  phase 1 (LoRA-A projection + RMSNorm): sharded over sequence (256 rows/core),
    result transposed to feature-major and AllGathered (bf16 latents).
    The kv+rope latent columns are computed first (kt-streaming loop with
    4-ktile batched weight DMAs) and gathered in an early collective that
    overlaps the q-latent loop. A tiny warm-up AllGather issued at kernel
    start absorbs the runtime's first-collective barrier (~50us) so the kv
    gather starts at its trigger. The q latents are gathered in two 768-col
    chunks so the q up-projection can start on chunk 0 while chunk 1 is in
    flight. Latent stores ride the scalar HWDGE ring so the q weight stream
    (sync ring) never queues behind them. The RMS 1/rms is folded into the
    latent transposes as a diagonal stationary matrix.
  phase 2 (q/kv up-proj, attention, o_proj): sharded over heads (4 heads/core),
    w_o input-dim sharded; partial outputs (bf16) summed on the host (the
    all-reduce). The kv up-projection (k_nope/V) is interleaved with the
    chunked q up-projection to keep the PE busy during the q gather.

All heavy matmuls run in bf16 with fp32 PSUM accumulation.
Everything feature-major ("X^T" layout [feature, seq]) in phase 2 so no big
transposes are needed:
  scores^T[sk, sq] accumulated from k^T/q^T; softmax normalizer via ones-matmul;
  (A @ V)^T = matmul(lhsT=V_rowmajor, rhs=A^T); o_proj consumes (A@V)^T directly.
The rope halves of the q up-projection are packed in head pairs (one 128-row
stationary per pair); the scores' rope matmuls then read qTB_pair / a
duplicated k_pe tile at partition offset 64 for odd heads (PE tile_position).
Attention is software-pipelined: AV matmuls trail their scores by two tiles,
each pair's rowsum matmuls run as one same-bank batch (single ones-LDWEIGHTS),
1/rowsum is exp(-ln(x)) on ACT, and the renorm broadcast is a bf16 K=1 matmul
deferred by one (head, sq-block) pair.
The causal diagonal masks are generated on-device (gpsimd affine_select).
"""

import os
import sys
from contextlib import ExitStack

import numpy as np

for _p in ("/opt/trn_rl_repo", "/root/.axon_site/_ro/trn_rl_repo"):
    if os.path.isdir(_p) and _p not in sys.path:
        sys.path.insert(0, _p)

import ml_dtypes  # noqa: E402

import concourse.bacc as bacc  # noqa: E402
import concourse.bass as bass  # noqa: E402
import concourse.mybir as mybir  # noqa: E402
import concourse.tile as tile  # noqa: E402
from concourse import bass_isa  # noqa: E402
from concourse.bass_utils import run_bass_kernel_spmd  # noqa: E402
from concourse.masks import make_identity  # noqa: E402

# ---------------------------------------------------------------- constants
NCORES = 8
S = 2048
SL = S // NCORES  # 256 local rows in phase 1
HID = 4096
Q_LORA = 1536
KV_LORA = 512
ROPE = 64
C = Q_LORA + KV_LORA + ROPE  # 2112
CKV_R = KV_LORA + ROPE  # 576 kv+rope latent cols
NOPE = 128
V_DIM = 128
H = 32
HL = H // NCORES  # 4 local heads
NPAIR = HL // 2  # 2 head pairs (rope packing)
Q_HEAD = NOPE + ROPE  # 192
EPS = 1e-6
NEG = -1e9

F32 = mybir.dt.float32
BF16 = mybir.dt.bfloat16
U8 = mybir.dt.uint8

CQ_TILES = Q_LORA // 128  # 12
CQ_HALF = CQ_TILES // 2  # 6 (gather chunk size)
CKV_TILES = KV_LORA // 128  # 4
HT_TILES = HID // 128  # 32
S_TILES = S // 128  # 16
SQB = 512
NSQB = S // SQB  # 4
EB = 512
NEB = HID // EB  # 8


# ---------------------------------------------------------------- program
def build_program() -> bass.Bass:
    nc = bacc.Bacc(
        "TRN2",
        target_bir_lowering=False,
        debug=False,
        num_devices=NCORES,
    )

    hid_d = nc.declare_dram_parameter("hid", [SL, HID], BF16, isOutput=False)
    wa_d = nc.declare_dram_parameter("wa", [HID, C], BF16, isOutput=False)
    wqb_d = nc.declare_dram_parameter("wqb", [Q_LORA, HL * Q_HEAD], BF16, isOutput=False)
    wkvb_d = nc.declare_dram_parameter(
        "wkvb", [KV_LORA, HL * (NOPE + V_DIM)], BF16, isOutput=False
    )
    wo_d = nc.declare_dram_parameter("wo", [HL * V_DIM, HID], BF16, isOutput=False)
    out_d = nc.declare_dram_parameter("out", [S, HID], BF16, isOutput=True)

    # collective bounce buffers (internal DRAM)
    cc_in_kv = nc.dram_tensor("cc_in_kv", [CKV_R, SL], BF16)
    cc_out_kv = nc.dram_tensor("cc_out_kv", [NCORES, CKV_R, SL], BF16, addr_space="Shared")
    cc_in_q0 = nc.dram_tensor("cc_in_q0", [CQ_HALF * 128, SL], BF16)
    cc_out_q0 = nc.dram_tensor(
        "cc_out_q0", [NCORES, CQ_HALF * 128, SL], BF16, addr_space="Shared"
    )
    cc_in_q1 = nc.dram_tensor("cc_in_q1", [CQ_HALF * 128, SL], BF16)
    cc_out_q1 = nc.dram_tensor(
        "cc_out_q1", [NCORES, CQ_HALF * 128, SL], BF16, addr_space="Shared"
    )

    with tile.TileContext(nc, num_cores=NCORES) as tc, ExitStack() as stack:
        # ---------------- small persistent constants (gpsimd, pre-warmup)
        misc = stack.enter_context(tc.tile_pool(name="misc", bufs=1))
        ident = misc.tile([128, 128], BF16, tag="ident", name="ident")
        make_identity(nc, ident[:])
        ones_sb = misc.tile([128, 1], BF16, tag="ones", name="ones")
        nc.gpsimd.memset(ones_sb[:], 1.0)
        mask_sb = misc.tile([128, 4 * SQB], F32, tag="mask", name="mask")
        eps_sb = misc.tile([128, 1], F32, tag="eps", name="eps")
        nc.gpsimd.memset(eps_sb[:], EPS)
        onesr_sb = misc.tile([1, 128], BF16, tag="onesr", name="onesr")
        nc.gpsimd.memset(onesr_sb[:], 1.0)
        # causal diagonal masks generated on-device, before the collectives
        # claim the gpsimd queue (a collective trigger blocks its engine until
        # the collective completes)
        nc.gpsimd.memset(mask_sb[:], 0.0)
        for dd in range(4):
            nc.gpsimd.affine_select(
                out=mask_sb[:, dd * SQB : (dd + 1) * SQB],
                in_=mask_sb[:, dd * SQB : (dd + 1) * SQB],
                compare_op=mybir.AluOpType.is_ge,
                fill=NEG,
                base=-128 * dd,
                # keep (0) where f - p - 128*d >= 0 else NEG
                pattern=[[1, SQB]],
                channel_multiplier=-1,
            )

        # PE warm-up burst: ~5us of dummy matmuls while the first DMAs land,
        # so the HAM activity monitor lifts the 4/8 (half-clock) gate before
        # the real phase-1 matmuls start (saves their cold-clock penalty)
        with ExitStack() as wu:
            wu_pool = wu.enter_context(tc.tile_pool(name="wup", bufs=2, space="PSUM"))
            wu_sink = misc.tile([1, 128], F32, tag="wusink", name="wusink")
            wps = None
            for _ in range(64):
                wps = wu_pool.tile([128, 128], F32, tag="wup", name="wps")
                nc.tensor.matmul(wps[:], ident[:], ident[:], start=True, stop=True)
            nc.vector.tensor_copy(wu_sink[:], wps[0:1, :])

        # phase-2 weights + kv latents: allocated up front; DMAs issued at the
        # point in each engine's stream where their data is needed next
        wkvb_pool = stack.enter_context(tc.tile_pool(name="wkvb", bufs=1))
        wkvb_sb = [
            wkvb_pool.tile(
                [128, HL * (NOPE + V_DIM)], BF16, tag=f"wkvb{kt}", name=f"wkvb{kt}"
            )
            for kt in range(CKV_TILES)
        ]
        wqb_pool = stack.enter_context(tc.tile_pool(name="wqb", bufs=1))
        wqb_sb = [
            wqb_pool.tile([128, HL * Q_HEAD], BF16, tag=f"wqb{kt}", name=f"wqb{kt}")
            for kt in range(CQ_TILES)
        ]
        latkv = stack.enter_context(tc.tile_pool(name="latkv", bufs=1))
        latkv_sb = [
            latkv.tile([128, S], BF16, tag=f"latkv{i}", name=f"latkv{i}")
            for i in range(CKV_TILES)
        ]
        # k_pe duplicated into both partition halves so odd heads can read it
        # (and their rope-q) at partition offset 64
        kpe2 = latkv.tile([128, S], BF16, tag="kpe2", name="kpe2")

        # ---------------- phase 1: a-projection on local rows (kt-streaming)
        with ExitStack() as p1:
            p1_pool = p1.enter_context(tc.tile_pool(name="p1", bufs=1))
            hid_sb = [
                p1_pool.tile([128, HID], BF16, tag=f"hid{s2}", name=f"hid{s2}")
                for s2 in range(2)
            ]
            # hid on the scalar ring so the wa weight stream (sync ring)
            # starts at t=0 in parallel
            for half in range(2):
                for s2 in range(2):
                    nc.scalar.dma_start(
                        hid_sb[s2][:, half * 2048 : (half + 1) * 2048],
                        hid_d[s2 * 128 : (s2 + 1) * 128, half * 2048 : (half + 1) * 2048],
                    )
            hidT = [
                p1_pool.tile([128, SL], BF16, tag=f"hidT{ht}", name=f"hidT{ht}")
                for ht in range(HT_TILES)
            ]
            lat_sb = [
                p1_pool.tile([128, C], BF16, tag=f"lat{s2}", name=f"lat{s2}")
                for s2 in range(2)
            ]
            stat = p1_pool.tile([128, 24], F32, tag="stat", name="stat")
            rms_scratch = p1_pool.tile([128, 512], F32, tag="rmssc", name="rmssc")
            diag_sb = [
                p1_pool.tile([128, 128], BF16, tag=f"diag{s2}", name=f"diag{s2}")
                for s2 in range(2)
            ]
            # local latents^T staging (feature-major, [*, SL])
            latTq_all = p1_pool.tile(
                [128, CQ_TILES * SL], BF16, tag="latTqa", name="latTqa"
            )
            latTkv_all = p1_pool.tile(
                [128, CKV_TILES * SL], BF16, tag="latTkva", name="latTkva"
            )
            latTkv_rope = p1_pool.tile([ROPE, SL], BF16, tag="latTkvr", name="latTkvr")

            kvw_pool = p1.enter_context(tc.tile_pool(name="kvw", bufs=8))
            qw_pool = p1.enter_context(tc.tile_pool(name="qw", bufs=8))

            # ---- kv + rope latents first (two 288-col psum accumulators x 2 s2)
            KVG = 4  # kt per kv weight DMA
            wa_kv_view = wa_d[:, Q_LORA : Q_LORA + CKV_R].rearrange(
                "(k p) c -> p k c", p=128
            )
            with ExitStack() as sA:
                tps_pool = sA.enter_context(
                    tc.tile_pool(name="tpsA", bufs=4, space="PSUM")
                )
                acc_pool = sA.enter_context(
                    tc.tile_pool(name="accA", bufs=1, space="PSUM")
                )

                def transpose_hid(ht):
                    for s2 in range(2):
                        pt = tps_pool.tile([128, 128], BF16, tag="tps", name="tps")
                        nc.tensor.transpose(
                            pt[:], hid_sb[s2][:, ht * 128 : (ht + 1) * 128], ident[:]
                        )
                        nc.vector.tensor_copy(
                            hidT[ht][:, s2 * 128 : (s2 + 1) * 128], pt[:]
                        )

                kv_acc = [
                    [
                        acc_pool.tile(
                            [128, 288], F32, tag=f"kvacc{s2}{b}", name=f"kvacc{s2}{b}"
                        )
                        for b in range(2)
                    ]
                    for s2 in range(2)
                ]
                transpose_hid(0)
                transpose_hid(1)
                for g in range(HT_TILES // KVG):
                    t = kvw_pool.tile(
                        [128, KVG * CKV_R], BF16, tag="kvw", name=f"kvw{g}"
                    )
                    nc.sync.dma_start(
                        t[:].rearrange("p (k c) -> p k c", k=KVG),
                        wa_kv_view[:, g * KVG : (g + 1) * KVG],
                    )
                    for j in range(KVG):
                        kt = g * KVG + j
                        if kt + 2 < HT_TILES:
                            transpose_hid(kt + 2)
                        for s2 in range(2):
                            for b in range(2):
                                nc.tensor.matmul(
                                    kv_acc[s2][b][:],
                                    hidT[kt][:, s2 * 128 : (s2 + 1) * 128],
                                    t[:, j * CKV_R + b * 288 : j * CKV_R + (b + 1) * 288],
                                    start=(kt == 0),
                                    stop=(kt == HT_TILES - 1),
                                )
                for s2 in range(2):
                    for b in range(2):
                        nc.scalar.copy(
                            lat_sb[s2][:, Q_LORA + b * 288 : Q_LORA + (b + 1) * 288],
                            kv_acc[s2][b][:],
                        )

            # wkvb prefetch on the scalar ring (executes ~kv-copy time, well
            # before the kv up-projection needs it)
            for kt in range(CKV_TILES):
                nc.scalar.dma_start(
                    wkvb_sb[kt][:], wkvb_d[kt * 128 : (kt + 1) * 128, :]
                )

            def rms_diag(col0, ncols, stat_base):
                """1/rms of lat_sb[:, col0:col0+ncols] per row, folded into a
                per-s2 diagonal matrix applied by the latent transposes."""
                nch = (ncols + 511) // 512
                for s2 in range(2):
                    for ch in range(nch):
                        w = min(512, ncols - ch * 512)
                        src = lat_sb[s2][:, col0 + ch * 512 : col0 + ch * 512 + w]
                        nc.vector.scalar_tensor_tensor(
                            rms_scratch[:, 0:w],
                            src,
                            1.0,
                            src,
                            op0=mybir.AluOpType.mult,
                            op1=mybir.AluOpType.mult,
                            accum_out=stat[:, stat_base + 3 * s2 + ch
                                           : stat_base + 3 * s2 + ch + 1],
                        )
                    for ch in range(1, nch):
                        nc.vector.tensor_add(
                            stat[:, stat_base + 3 * s2 : stat_base + 3 * s2 + 1],
                            stat[:, stat_base + 3 * s2 : stat_base + 3 * s2 + 1],
                            stat[:, stat_base + 3 * s2 + ch
                                 : stat_base + 3 * s2 + ch + 1],
                        )
                for s2 in range(2):
                    nc.scalar.activation(
                        stat[:, stat_base + 6 + s2 : stat_base + 7 + s2],
                        stat[:, stat_base + 3 * s2 : stat_base + 3 * s2 + 1],
                        mybir.ActivationFunctionType.Sqrt,
                        scale=1.0 / ncols,
                        bias=eps_sb[:],
                    )
                for s2 in range(2):
                    nc.vector.reciprocal(
                        stat[:, stat_base + 8 + s2 : stat_base + 9 + s2],
                        stat[:, stat_base + 6 + s2 : stat_base + 7 + s2],
                    )
                for s2 in range(2):
                    nc.vector.tensor_scalar_mul(
                        diag_sb[s2][:],
                        ident[:],
                        stat[:, stat_base + 8 + s2 : stat_base + 9 + s2],
                    )

            def transpose_lat(src_col, w, dst, tps_pool, scaled):
                """dst[:, s2*128...] = (lat_sb[s2][:, src_col:src_col+w])T,
                optionally scaled per seq row (lat.T @ diag(1/rms))."""
                for s2 in range(2):
                    pt = tps_pool.tile([128, 128], F32, tag="tps", name="tpsl")
                    tmat = diag_sb[s2] if scaled else ident
                    nc.tensor.matmul(
                        pt[:w, :],
                        lat_sb[s2][:, src_col : src_col + w],
                        tmat[:],
                        start=True,
                        stop=True,
                    )
                    nc.vector.tensor_copy(
                        dst[:, s2 * 128 : (s2 + 1) * 128], pt[:w, :]
                    )

            with ExitStack() as sB:
                tpsB = sB.enter_context(tc.tile_pool(name="tpsB", bufs=2, space="PSUM"))
                rms_diag(Q_LORA, KV_LORA, 0)
                for ct in range(CKV_TILES):
                    transpose_lat(
                        Q_LORA + ct * 128,
                        128,
                        latTkv_all[:, ct * SL : (ct + 1) * SL],
                        tpsB,
                        scaled=True,
                    )
                transpose_lat(Q_LORA + KV_LORA, ROPE, latTkv_rope[:], tpsB, scaled=False)
                # latent stores on the scalar ring: the q weight stream on the
                # sync ring must not queue behind them
                nc.scalar.dma_start(
                    cc_in_kv[0:KV_LORA].rearrange("(ct p) s -> p ct s", p=128),
                    latTkv_all[:].rearrange("p (ct s) -> p ct s", ct=CKV_TILES),
                )
                nc.scalar.dma_start(
                    cc_in_kv[KV_LORA : KV_LORA + ROPE, :], latTkv_rope[:]
                )
            nc.gpsimd.collective_compute(
                "AllGather",
                mybir.AluOpType.bypass,
                replica_groups=[list(range(NCORES))],
                ins=[cc_in_kv[:].opt()],
                outs=[cc_out_kv[:].opt()],
            )

            # ---- q latents (three 512-col psum accumulators x 2 s2)
            with ExitStack() as sC:
                accC = sC.enter_context(tc.tile_pool(name="accC", bufs=1, space="PSUM"))
                q_acc = [
                    [
                        accC.tile(
                            [128, 512], F32, tag=f"qacc{s2}{b}", name=f"qacc{s2}{b}"
                        )
                        for b in range(3)
                    ]
                    for s2 in range(2)
                ]
                QG = 2  # kt per q weight DMA
                wa_q_view = wa_d[:, 0:Q_LORA].rearrange("(k p) c -> p k c", p=128)
                for g in range(HT_TILES // QG):
                    t = qw_pool.tile(
                        [128, QG * Q_LORA], BF16, tag="qw", name=f"qw{g}"
                    )
                    nc.sync.dma_start(
                        t[:].rearrange("p (k c) -> p k c", k=QG),
                        wa_q_view[:, g * QG : (g + 1) * QG],
                    )
                    for j in range(QG):
                        kt = g * QG + j
                        for s2 in range(2):
                            for b in range(3):
                                nc.tensor.matmul(
                                    q_acc[s2][b][:],
                                    hidT[kt][:, s2 * 128 : (s2 + 1) * 128],
                                    t[:, j * Q_LORA + b * 512 : j * Q_LORA + (b + 1) * 512],
                                    start=(kt == 0),
                                    stop=(kt == HT_TILES - 1),
                                )
                for s2 in range(2):
                    for b in range(3):
                        nc.scalar.copy(
                            lat_sb[s2][:, b * 512 : (b + 1) * 512], q_acc[s2][b][:]
                        )

            # wqb prefetch on the sync ring right behind the q weight stream
            for kt in range(CQ_TILES):
                nc.sync.dma_start(wqb_sb[kt][:], wqb_d[kt * 128 : (kt + 1) * 128, :])

            with ExitStack() as sD:
                tpsD = sD.enter_context(tc.tile_pool(name="tpsD", bufs=2, space="PSUM"))
                rms_diag(0, Q_LORA, 12)
                # chunk 0: ct 0-5 -> store -> gather; chunk 1: ct 6-11
                for ct in range(CQ_HALF):
                    transpose_lat(
                        ct * 128,
                        128,
                        latTq_all[:, ct * SL : (ct + 1) * SL],
                        tpsD,
                        scaled=True,
                    )
                nc.scalar.dma_start(
                    cc_in_q0[:].rearrange("(ct p) s -> p ct s", p=128),
                    latTq_all[:, 0 : CQ_HALF * SL].rearrange(
                        "p (ct s) -> p ct s", ct=CQ_HALF
                    ),
                )
                for ct in range(CQ_HALF, CQ_TILES):
                    transpose_lat(
                        ct * 128,
                        128,
                        latTq_all[:, ct * SL : (ct + 1) * SL],
                        tpsD,
                        scaled=True,
                    )
                nc.scalar.dma_start(
                    cc_in_q1[:].rearrange("(ct p) s -> p ct s", p=128),
                    latTq_all[:, CQ_HALF * SL :].rearrange(
                        "p (ct s) -> p ct s", ct=CQ_HALF
                    ),
                )
            nc.gpsimd.collective_compute(
                "AllGather",
                mybir.AluOpType.bypass,
                replica_groups=[list(range(NCORES))],
                ins=[cc_in_q0[:].opt()],
                outs=[cc_out_q0[:].opt()],
            )
            nc.gpsimd.collective_compute(
                "AllGather",
                mybir.AluOpType.bypass,
                replica_groups=[list(range(NCORES))],
                ins=[cc_in_q1[:].opt()],
                outs=[cc_out_q1[:].opt()],
            )

            # gathered kv latents into SBUF. The sync/scalar rings are idle
            # once phase 1 drains and (unlike gpsimd) not blocked by the
            # in-flight q collectives; each entry's gate time is later than
            # the previous entry's, so no head-of-line blocking.
            cc_kv_view = cc_out_kv[:].rearrange("j c s -> c j s")
            for i in range(CKV_TILES):
                eng = nc.sync if i < 2 else nc.scalar
                eng.dma_start(
                    latkv_sb[i][:].rearrange("c (j s) -> c j s", j=NCORES),
                    cc_kv_view[i * 128 : (i + 1) * 128],
                )
            for half in range(2):
                nc.scalar.dma_start(
                    kpe2[half * 64 : (half + 1) * 64, :].rearrange(
                        "c (j s) -> c j s", j=NCORES
                    ),
                    cc_kv_view[KV_LORA : KV_LORA + ROPE],
                )

        # ---------------- phase 2
        kvpool = stack.enter_context(tc.tile_pool(name="kvpool", bufs=1))
        knopeT = [
            kvpool.tile([128, S], BF16, tag=f"knopeT{h}", name=f"knopeT{h}")
            for h in range(HL)
        ]
        v_sb = [
            kvpool.tile([128, HL * V_DIM], BF16, tag=f"v{st}", name=f"v{st}")
            for st in range(S_TILES)
        ]
        qT = stack.enter_context(tc.tile_pool(name="qT", bufs=1))
        qTA = [qT.tile([128, S], BF16, tag=f"qTA{h}", name=f"qTA{h}") for h in range(HL)]
        # rope q of head pair (2p, 2p+1) stacked in partition halves
        qTB = [qT.tile([128, S], BF16, tag=f"qTB{p}", name=f"qTB{p}") for p in range(NPAIR)]
        outT_pool = stack.enter_context(tc.tile_pool(name="outT", bufs=1))
        outT = [
            outT_pool.tile([128, S], BF16, tag=f"outT{h}", name=f"outT{h}")
            for h in range(HL)
        ]

        # kv up-proj interleaved with the chunked q up-proj (one scope so the
        # PSUM pools coexist: pkv 2 banks + pq 6 banks = 8)
        with ExitStack() as p2q:
            latq = p2q.enter_context(tc.tile_pool(name="latq", bufs=1))
            latq_sb = [
                latq.tile([128, S], BF16, tag=f"latq{ct}", name=f"latq{ct}")
                for ct in range(CQ_TILES)
            ]
            cc_q_views = [
                cc_out_q0[:].rearrange("j c s -> c j s"),
                cc_out_q1[:].rearrange("j c s -> c j s"),
            ]
            # gpsimd is blocked until the q collectives complete, so chunk-0
            # loads ride the sync/scalar rings only; chunk-1 loads (gated on
            # the last gather, exactly when gpsimd unblocks) use all three
            for ct in range(CQ_TILES):
                half, cth = divmod(ct, CQ_HALF)
                if half == 0:
                    eng = (nc.sync, nc.scalar)[ct % 2]
                else:
                    eng = (nc.sync, nc.scalar, nc.gpsimd)[ct % 3]
                eng.dma_start(
                    latq_sb[ct][:].rearrange("c (j s) -> c j s", j=NCORES),
                    cc_q_views[half][cth * 128 : (cth + 1) * 128],
                )
            pkv_pool = p2q.enter_context(tc.tile_pool(name="pkv", bufs=2, space="PSUM"))
            pq_pool = p2q.enter_context(tc.tile_pool(name="pq", bufs=1, space="PSUM"))

            def knope_upproj():
                for h in range(HL):
                    for skb in range(NSQB):
                        pk = pkv_pool.tile([128, SQB], F32, tag="pkv", name="pk")
                        for kt in range(CKV_TILES):
                            nc.tensor.matmul(
                                pk[:],
                                wkvb_sb[kt][
                                    :, h * (NOPE + V_DIM) : h * (NOPE + V_DIM) + NOPE
                                ],
                                latkv_sb[kt][:, skb * SQB : (skb + 1) * SQB],
                                start=(kt == 0),
                                stop=(kt == CKV_TILES - 1),
                            )
                        nc.vector.tensor_copy(
                            knopeT[h][:, skb * SQB : (skb + 1) * SQB], pk[:]
                        )

            def v_upproj(st_range):
                for st in st_range:
                    pv = pkv_pool.tile([128, HL * V_DIM], F32, tag="pkv", name="pv")
                    for kt in range(CKV_TILES):
                        rhs = wkvb_sb[kt][:].rearrange(
                            "c (h d) -> c h d", h=HL
                        )[:, :, NOPE:]
                        nc.tensor.matmul(
                            pv[:],
                            latkv_sb[kt][:, st * 128 : (st + 1) * 128],
                            rhs,
                            start=(kt == 0),
                            stop=(kt == CKV_TILES - 1),
                        )
                    nc.vector.tensor_copy(v_sb[st][:], pv[:])

            def q_upproj_block(sqb, pqs, kts):
                # wqb cols are host-reordered: [nope h0..h3 | rope h0..h3]
                for kt in kts:
                    for h in range(HL):
                        nc.tensor.matmul(
                            pqs[h][:],
                            wqb_sb[kt][:, h * NOPE : (h + 1) * NOPE],
                            latq_sb[kt][:, sqb * SQB : (sqb + 1) * SQB],
                            start=(kt == 0),
                            stop=(kt == CQ_TILES - 1),
                        )
                    for p in range(NPAIR):
                        nc.tensor.matmul(
                            pqs[HL + p][:],
                            wqb_sb[kt][
                                :, HL * NOPE + p * 128 : HL * NOPE + (p + 1) * 128
                            ],
                            latq_sb[kt][:, sqb * SQB : (sqb + 1) * SQB],
                            start=(kt == 0),
                            stop=(kt == CQ_TILES - 1),
                        )

            def q_copies(sqb, pqs):
                for h in range(HL):
                    nc.scalar.copy(
                        qTA[h][:, sqb * SQB : (sqb + 1) * SQB], pqs[h][:]
                    )
                for p in range(NPAIR):
                    nc.scalar.copy(
                        qTB[p][:, sqb * SQB : (sqb + 1) * SQB], pqs[HL + p][:]
                    )

            def q_psums():
                return [
                    pq_pool.tile([128, SQB], F32, tag=f"pq{u}", name=f"pq{u}")
                    for u in range(HL + NPAIR)
                ]

            # emission order tuned to data arrival: kv work (gated on the kv
            # gather) fills the q-gather window; V's second half fills the
            # chunk-0 -> chunk-1 gap
            knope_upproj()
            v_upproj(range(0, 8))
            pqs0 = q_psums()
            q_upproj_block(0, pqs0, range(0, CQ_HALF))
            v_upproj(range(8, S_TILES))
            q_upproj_block(0, pqs0, range(CQ_HALF, CQ_TILES))
            q_copies(0, pqs0)
            for sqb in range(1, NSQB):
                pqs = q_psums()
                q_upproj_block(sqb, pqs, range(0, CQ_HALF))
                q_upproj_block(sqb, pqs, range(CQ_HALF, CQ_TILES))
                q_copies(sqb, pqs)

        # o_proj weights: loaded late (SBUF freed by the q latents)
        wo_pool = stack.enter_context(tc.tile_pool(name="wo", bufs=1))
        wo_sb = [
            wo_pool.tile([128, HID], BF16, tag=f"wo{h}", name=f"wo{h}")
            for h in range(HL)
        ]
        for h in range(HL):
            nc.sync.dma_start(wo_sb[h][:], wo_d[h * 128 : (h + 1) * 128, :])

        # ---------------- attention (causal, block-skipped) + interleaved o_proj
        # bq-outer so each 512-row sq block's outT completes early; its o_proj
        # block is emitted as soon as the last head's epilogue drains, filling
        # attention-pipeline bubbles and spreading the output DMA.
        # Rowsums: DVE pair-adds halve the exp tiles (bf16), then one
        # accumulating ones-matmul run per (h, bq) over the nk/2 pair-sums
        # (half the PE stream cost of per-tile rowsum matmuls); 1/rowsum is
        # exp(-ln(x)) on ACT and the renorm broadcast a bf16 K=1 matmul,
        # both deferred as in the baseline pipeline.
        with ExitStack() as p2a:
            ps_pool = p2a.enter_context(tc.tile_pool(name="ps", bufs=5, space="PSUM"))
            psum_sum_pool = p2a.enter_context(
                tc.tile_pool(name="psums", bufs=1, space="PSUM")
            )
            psum_o_pool = p2a.enter_context(
                tc.tile_pool(name="psumo", bufs=2, space="PSUM")
            )
            a_pool = p2a.enter_context(tc.tile_pool(name="apool", bufs=10))
            apair_pool = p2a.enter_context(tc.tile_pool(name="apair", bufs=6))
            aquad_pool = p2a.enter_context(tc.tile_pool(name="aquad", bufs=6))
            bc_pool = p2a.enter_context(tc.tile_pool(name="bcpool", bufs=3))

            tile_q = []  # score tiles awaiting their AV matmuls
            ep_q = []  # pairs awaiting the renormalization epilogue

            def drain_tile():
                a, h, bq, tk, nk, po = tile_q.pop(0)
                nc.tensor.matmul(
                    po[:],
                    v_sb[tk][:, h * V_DIM : (h + 1) * V_DIM],
                    a[:],
                    start=(tk == 0),
                    stop=(tk == nk - 1),
                )

            def drain_sums(pair):
                h, bq, po, psum, pair_sums = pair
                for i, ap in enumerate(pair_sums):
                    nc.tensor.matmul(
                        psum[:],
                        ones_sb[:],
                        ap[:],
                        start=(i == 0),
                        stop=(i == len(pair_sums) - 1),
                    )
                # 1/rowsum on DVE (custom op, ~18 bits, rowsum > 0 always):
                # keeps Ln/Exp off the ACT engine, whose activation-table set
                # would thrash against the softmax Exp (1.3us reload per swap)
                rs32 = bc_pool.tile([1, SQB], F32, tag="rs32", name="rs32")
                nc.vector.reciprocal_approx_fast(rs32[:], psum[:])
                rs = bc_pool.tile([1, SQB], BF16, tag="rs", name="rs")
                nc.vector.tensor_copy(rs[:], rs32[:])
                ep_q.append((h, bq, po, rs))

            def drain_epilogue():
                h, bq, po, rs = ep_q.pop(0)
                bc_ps = ps_pool.tile([128, SQB], F32, tag="ps", name="bc_ps")
                nc.tensor.matmul(bc_ps[:], onesr_sb[:], rs[:], start=True, stop=True)
                bc_sb = bc_pool.tile([128, SQB], F32, tag="bc", name="bc_sb")
                nc.scalar.copy(bc_sb[:], bc_ps[:])
                nc.vector.tensor_mul(
                    outT[h][:, bq * SQB : (bq + 1) * SQB], po[:], bc_sb[:]
                )

            prev_pair = None
            for bq in range(NSQB):
                nk = 4 * (bq + 1)
                for h in range(HL):
                    off = 64 * (h % 2)
                    qTBh = qTB[h // 2]
                    po = psum_o_pool.tile([128, SQB], F32, tag="psumo", name="po")
                    psum = psum_sum_pool.tile([1, SQB], F32, tag="psums", name="psum")
                    pair_sums = []
                    pend_a = None
                    pend_pair = None
                    for tk in range(nk):
                        ps = ps_pool.tile([128, SQB], F32, tag="ps", name="ps")
                        nc.tensor.matmul(
                            ps[:],
                            knopeT[h][:, tk * 128 : (tk + 1) * 128],
                            qTA[h][:, bq * SQB : (bq + 1) * SQB],
                            start=True,
                            stop=False,
                        )
                        nc.tensor.matmul(
                            ps[:],
                            kpe2[off : off + 64, tk * 128 : (tk + 1) * 128],
                            qTBh[off : off + 64, bq * SQB : (bq + 1) * SQB],
                            start=False,
                            stop=True,
                        )
                        d = tk - 4 * bq
                        if d >= 0:
                            nc.vector.tensor_add(
                                ps[:], ps[:], mask_sb[:, d * SQB : (d + 1) * SQB]
                            )
                        a = a_pool.tile([128, SQB], BF16, tag="a", name="a")
                        nc.scalar.activation(
                            a[:], ps[:], mybir.ActivationFunctionType.Exp
                        )
                        # rowsum pre-reduction: two DVE add levels (bf16) so
                        # each (h, bq) needs only nk/4 ones-matmul streams
                        if tk % 2 == 0:
                            pend_a = a
                        else:
                            apair = apair_pool.tile(
                                [128, SQB], BF16, tag="apair", name="apair"
                            )
                            nc.vector.tensor_add(apair[:], pend_a[:], a[:])
                            if tk % 4 == 1:
                                pend_pair = apair
                            else:
                                aquad = aquad_pool.tile(
                                    [128, SQB], BF16, tag="aquad", name="aquad"
                                )
                                nc.vector.tensor_add(
                                    aquad[:], pend_pair[:], apair[:]
                                )
                                pair_sums.append(aquad)
                        tile_q.append((a, h, bq, tk, nk, po))
                        while len(tile_q) > 3:
                            drain_tile()
                        if tk == 2 and prev_pair is not None:
                            drain_sums(prev_pair)
                            prev_pair = None
                        while len(ep_q) > 1:
                            drain_epilogue()
                    prev_pair = (h, bq, po, psum, pair_sums)
            while tile_q:
                drain_tile()
            if prev_pair is not None:
                drain_sums(prev_pair)
            while ep_q:
                drain_epilogue()

        # ---------------- o_proj (partial: summed across cores on host)
        # kept as a dedicated tail phase: its matmuls have trivially-satisfied
        # dependencies there and pipeline back-to-back on the PE
        with ExitStack() as p2o:
            pe_pool = p2o.enter_context(tc.tile_pool(name="pe", bufs=4, space="PSUM"))
            stage_pool = p2o.enter_context(tc.tile_pool(name="stage", bufs=3))
            for st in range(S_TILES):
                for half in range(2):
                    stg = stage_pool.tile([128, 4 * EB], BF16, tag="stage", name="stg")
                    for ebl in range(4):
                        eb = half * 4 + ebl
                        pe = pe_pool.tile([128, EB], F32, tag="pe", name="pe")
                        for h in range(HL):
                            nc.tensor.matmul(
                                pe[:],
                                outT[h][:, st * 128 : (st + 1) * 128],
                                wo_sb[h][:, eb * EB : (eb + 1) * EB],
                                start=(h == 0),
                                stop=(h == HL - 1),
                            )
                        nc.vector.tensor_copy(
                            stg[:, ebl * EB : (ebl + 1) * EB], pe[:]
                        )
                    nc.gpsimd.dma_start(
                        out_d[
                            st * 128 : (st + 1) * 128,
                            half * 4 * EB : (half + 1) * 4 * EB,
                        ],
                        stg[:],
                    )

    nc.compile()
    return nc


_PROGRAM_CACHE = {}


def _get_program() -> bass.Bass:
    if "nc" not in _PROGRAM_CACHE:
        _PROGRAM_CACHE["nc"] = build_program()
    return _PROGRAM_CACHE["nc"]


def prepare_inputs(
    hidden_states, w_qkv_a, q_a_gamma, w_q_b, kv_a_gamma, w_kv_b, w_o, b_o
):
    """Host-side prep: fold gammas + attention scale into B weights, cast to
    bf16, slice per core."""
    bf = ml_dtypes.bfloat16
    hs = np.asarray(hidden_states, np.float32).reshape(S, HID)
    scale = float(Q_HEAD) ** -0.5
    wqb_eff = (
        np.asarray(w_q_b, np.float32)
        * np.asarray(q_a_gamma, np.float32)[:, None]
        * scale
    )
    wkvb_eff = (
        np.asarray(w_kv_b, np.float32) * np.asarray(kv_a_gamma, np.float32)[:, None]
    )
    wa_bf = np.asarray(w_qkv_a, np.float32).astype(bf)
    hs_bf = hs.astype(bf)

    wqb_r = wqb_eff.reshape(Q_LORA, H, Q_HEAD)
    wkvb_r = wkvb_eff.reshape(KV_LORA, H, NOPE + V_DIM)
    wo_r = np.asarray(w_o, np.float32).reshape(H, V_DIM, HID)

    in_maps = []
    for c in range(NCORES):
        hsl = np.ascontiguousarray(hs_bf[c * SL : (c + 1) * SL])
        wqb_loc = wqb_r[:, c * HL : (c + 1) * HL]  # [Q_LORA, HL, Q_HEAD]
        # column order: [nope h0..h3 | rope h0..h3] (pair-packed rope)
        wqb_c = np.ascontiguousarray(
            np.concatenate(
                [
                    wqb_loc[:, :, :NOPE].reshape(Q_LORA, HL * NOPE),
                    wqb_loc[:, :, NOPE:].reshape(Q_LORA, HL * ROPE),
                ],
                axis=1,
            ).astype(bf)
        )
        wkvb_c = np.ascontiguousarray(
            wkvb_r[:, c * HL : (c + 1) * HL]
            .reshape(KV_LORA, HL * (NOPE + V_DIM))
            .astype(bf)
        )
        wo_c = np.ascontiguousarray(
            wo_r[c * HL : (c + 1) * HL].reshape(HL * V_DIM, HID).astype(bf)
        )
        in_maps.append(
            {
                "hid": hsl,
                "wa": wa_bf,
                "wqb": wqb_c,
                "wkvb": wkvb_c,
                "wo": wo_c,
            }
        )
    return in_maps


def kernel(**inputs) -> np.ndarray:
    in_maps = prepare_inputs(**inputs)
    nc = _get_program()
    res = run_bass_kernel_spmd(nc, in_maps, list(range(NCORES)))
    out = np.zeros((S, HID), np.float64)
    for r in res.results:
        out += np.asarray(r["out"], np.float32)
    out = out.astype(np.float32) + np.asarray(inputs["b_o"], np.float32)[None, :]
    return out.reshape(1, S, HID)


# revision 34
# speedup vs baseline: 1.0257x; 1.0120x over previous
"""
MLA attention (DeepSeek-style) on 8 TRN2 NeuronCores.

Sharding:
  phase 1 (LoRA-A projection + RMSNorm): sharded over sequence (256 rows/core),
    result transposed to feature-major and AllGathered (bf16 latents).
    The kv+rope latent columns are computed first (kt-streaming loop with
    4-ktile batched weight DMAs) and gathered in an early collective that
    overlaps the q-latent loop. A tiny warm-up AllGather issued at kernel
    start absorbs the runtime's first-collective barrier (~50us) so the kv
    gather starts at its trigger. The q latents are gathered in two 768-col
    chunks so the q up-projection can start on chunk 0 while chunk 1 is in
    flight. Latent stores ride the scalar HWDGE ring so the q weight stream
    (sync ring) never queues behind them. The RMS 1/rms is folded into the
    latent transposes as a diagonal stationary matrix.
  phase 2 (q/kv up-proj, attention, o_proj): sharded over heads (4 heads/core),
    w_o input-dim sharded; partial outputs (bf16) summed on the host (the
    all-reduce). The kv up-projection (k_nope/V) is interleaved with the
    chunked q up-projection to keep the PE busy during the q gather.

All heavy matmuls run in bf16 with fp32 PSUM accumulation.
Everything feature-major ("X^T" layout [feature, seq]) in phase 2 so no big
transposes are needed:
  scores^T[sk, sq] accumulated from k^T/q^T; softmax normalizer via ones-matmul;
  (A @ V)^T = matmul(lhsT=V_rowmajor, rhs=A^T); o_proj consumes (A@V)^T directly.
The rope halves of the q up-projection are packed in head pairs (one 128-row
stationary per pair); the scores' rope matmuls then read qTB_pair / a
duplicated k_pe tile at partition offset 64 for odd heads (PE tile_position).
Attention is software-pipelined: AV matmuls trail their scores by two tiles,
each pair's rowsum matmuls run as one same-bank batch (single ones-LDWEIGHTS),
1/rowsum is exp(-ln(x)) on ACT, and the renorm broadcast is a bf16 K=1 matmul
deferred by one (head, sq-block) pair.
The causal diagonal masks are generated on-device (gpsimd affine_select).
"""

import os
import sys
from contextlib import ExitStack

import numpy as np

for _p in ("/opt/trn_rl_repo", "/root/.axon_site/_ro/trn_rl_repo"):
    if os.path.isdir(_p) and _p not in sys.path:
        sys.path.insert(0, _p)

import ml_dtypes  # noqa: E402

import concourse.bacc as bacc  # noqa: E402
import concourse.bass as bass  # noqa: E402
import concourse.mybir as mybir  # noqa: E402
import concourse.tile as tile  # noqa: E402
from concourse import bass_isa  # noqa: E402
from concourse.bass_utils import run_bass_kernel_spmd  # noqa: E402
from concourse.masks import make_identity  # noqa: E402

# ---------------------------------------------------------------- constants
NCORES = 8
S = 2048
SL = S // NCORES  # 256 local rows in phase 1
HID = 4096
Q_LORA = 1536
KV_LORA = 512
ROPE = 64
C = Q_LORA + KV_LORA + ROPE  # 2112
CKV_R = KV_LORA + ROPE  # 576 kv+rope latent cols
NOPE = 128
V_DIM = 128
H = 32
HL = H // NCORES  # 4 local heads
NPAIR = HL // 2  # 2 head pairs (rope packing)
Q_HEAD = NOPE + ROPE  # 192
EPS = 1e-6
NEG = -1e9

F32 = mybir.dt.float32
BF16 = mybir.dt.bfloat16
U8 = mybir.dt.uint8

CQ_TILES = Q_LORA // 128  # 12
CQ_HALF = CQ_TILES // 2  # 6 (gather chunk size)
CKV_TILES = KV_LORA // 128  # 4
HT_TILES = HID // 128  # 32
S_TILES = S // 128  # 16
SQB = 512
NSQB = S // SQB  # 4
EB = 512
NEB = HID // EB  # 8


# ---------------------------------------------------------------- program
def build_program() -> bass.Bass:
    nc = bacc.Bacc(
        "TRN2",
        target_bir_lowering=False,
        debug=False,
        num_devices=NCORES,
    )

    hid_d = nc.declare_dram_parameter("hid", [SL, HID], BF16, isOutput=False)
    wa_d = nc.declare_dram_parameter("wa", [HID, C], BF16, isOutput=False)
    wqb_d = nc.declare_dram_parameter("wqb", [Q_LORA, HL * Q_HEAD], BF16, isOutput=False)
    wkvb_d = nc.declare_dram_parameter(
        "wkvb", [KV_LORA, HL * (NOPE + V_DIM)], BF16, isOutput=False
    )
    wo_d = nc.declare_dram_parameter("wo", [HL * V_DIM, HID], BF16, isOutput=False)
    out_d = nc.declare_dram_parameter("out", [S, HID], BF16, isOutput=True)

    # collective bounce buffers (internal DRAM)
    cc_in_kv = nc.dram_tensor("cc_in_kv", [CKV_R, SL], BF16)
    cc_out_kv = nc.dram_tensor("cc_out_kv", [NCORES, CKV_R, SL], BF16, addr_space="Shared")
    cc_in_q0 = nc.dram_tensor("cc_in_q0", [CQ_HALF * 128, SL], BF16)
    cc_out_q0 = nc.dram_tensor(
        "cc_out_q0", [NCORES, CQ_HALF * 128, SL], BF16, addr_space="Shared"
    )
    cc_in_q1 = nc.dram_tensor("cc_in_q1", [CQ_HALF * 128, SL], BF16)
    cc_out_q1 = nc.dram_tensor(
        "cc_out_q1", [NCORES, CQ_HALF * 128, SL], BF16, addr_space="Shared"
    )

    with tile.TileContext(nc, num_cores=NCORES) as tc, ExitStack() as stack:
        # ---------------- small persistent constants (gpsimd, pre-warmup)
        misc = stack.enter_context(tc.tile_pool(name="misc", bufs=1))
        ident = misc.tile([128, 128], BF16, tag="ident", name="ident")
        make_identity(nc, ident[:])
        ones_sb = misc.tile([128, 1], BF16, tag="ones", name="ones")
        nc.gpsimd.memset(ones_sb[:], 1.0)
        mask_sb = misc.tile([128, 4 * SQB], F32, tag="mask", name="mask")
        eps_sb = misc.tile([128, 1], F32, tag="eps", name="eps")
        nc.gpsimd.memset(eps_sb[:], EPS)
        onesr_sb = misc.tile([1, 128], BF16, tag="onesr", name="onesr")
        nc.gpsimd.memset(onesr_sb[:], 1.0)
        # causal diagonal masks generated on-device, before the collectives
        # claim the gpsimd queue (a collective trigger blocks its engine until
        # the collective completes)
        nc.gpsimd.memset(mask_sb[:], 0.0)
        for dd in range(4):
            nc.gpsimd.affine_select(
                out=mask_sb[:, dd * SQB : (dd + 1) * SQB],
                in_=mask_sb[:, dd * SQB : (dd + 1) * SQB],
                compare_op=mybir.AluOpType.is_ge,
                fill=NEG,
                base=-128 * dd,
                # keep (0) where f - p - 128*d >= 0 else NEG
                pattern=[[1, SQB]],
                channel_multiplier=-1,
            )

        # PE warm-up burst: ~5us of dummy matmuls while the first DMAs land,
        # so the HAM activity monitor lifts the 4/8 (half-clock) gate before
        # the real phase-1 matmuls start (saves their cold-clock penalty)
        with ExitStack() as wu:
            wu_pool = wu.enter_context(tc.tile_pool(name="wup", bufs=2, space="PSUM"))
            wu_sink = misc.tile([1, 128], F32, tag="wusink", name="wusink")
            wps = None
            for _ in range(64):
                wps = wu_pool.tile([128, 128], F32, tag="wup", name="wps")
                nc.tensor.matmul(wps[:], ident[:], ident[:], start=True, stop=True)
            nc.vector.tensor_copy(wu_sink[:], wps[0:1, :])

        # phase-2 weights + kv latents: allocated up front; DMAs issued at the
        # point in each engine's stream where their data is needed next
        wkvb_pool = stack.enter_context(tc.tile_pool(name="wkvb", bufs=1))
        wkvb_sb = [
            wkvb_pool.tile(
                [128, HL * (NOPE + V_DIM)], BF16, tag=f"wkvb{kt}", name=f"wkvb{kt}"
            )
            for kt in range(CKV_TILES)
        ]
        wqb_pool = stack.enter_context(tc.tile_pool(name="wqb", bufs=1))
        wqb_sb = [
            wqb_pool.tile([128, HL * Q_HEAD], BF16, tag=f"wqb{kt}", name=f"wqb{kt}")
            for kt in range(CQ_TILES)
        ]
        latkv = stack.enter_context(tc.tile_pool(name="latkv", bufs=1))
        latkv_sb = [
            latkv.tile([128, S], BF16, tag=f"latkv{i}", name=f"latkv{i}")
            for i in range(CKV_TILES)
        ]
        # k_pe duplicated into both partition halves so odd heads can read it
        # (and their rope-q) at partition offset 64
        kpe2 = latkv.tile([128, S], BF16, tag="kpe2", name="kpe2")

        # ---------------- phase 1: a-projection on local rows (kt-streaming)
        with ExitStack() as p1:
            p1_pool = p1.enter_context(tc.tile_pool(name="p1", bufs=1))
            hid_sb = [
                p1_pool.tile([128, HID], BF16, tag=f"hid{s2}", name=f"hid{s2}")
                for s2 in range(2)
            ]
            # hid on the scalar ring so the wa weight stream (sync ring)
            # starts at t=0 in parallel
            for half in range(2):
                for s2 in range(2):
                    nc.scalar.dma_start(
                        hid_sb[s2][:, half * 2048 : (half + 1) * 2048],
                        hid_d[s2 * 128 : (s2 + 1) * 128, half * 2048 : (half + 1) * 2048],
                    )
            hidT = [
                p1_pool.tile([128, SL], BF16, tag=f"hidT{ht}", name=f"hidT{ht}")
                for ht in range(HT_TILES)
            ]
            lat_sb = [
                p1_pool.tile([128, C], BF16, tag=f"lat{s2}", name=f"lat{s2}")
                for s2 in range(2)
            ]
            stat = p1_pool.tile([128, 24], F32, tag="stat", name="stat")
            rms_scratch = p1_pool.tile([128, 512], F32, tag="rmssc", name="rmssc")
            diag_sb = [
                p1_pool.tile([128, 128], BF16, tag=f"diag{s2}", name=f"diag{s2}")
                for s2 in range(2)
            ]
            # local latents^T staging (feature-major, [*, SL])
            latTq_all = p1_pool.tile(
                [128, CQ_TILES * SL], BF16, tag="latTqa", name="latTqa"
            )
            latTkv_all = p1_pool.tile(
                [128, CKV_TILES * SL], BF16, tag="latTkva", name="latTkva"
            )
            latTkv_rope = p1_pool.tile([ROPE, SL], BF16, tag="latTkvr", name="latTkvr")

            kvw_pool = p1.enter_context(tc.tile_pool(name="kvw", bufs=8))
            qw_pool = p1.enter_context(tc.tile_pool(name="qw", bufs=8))

            # ---- kv + rope latents first (two 288-col psum accumulators x 2 s2)
            KVG = 4  # kt per kv weight DMA
            wa_kv_view = wa_d[:, Q_LORA : Q_LORA + CKV_R].rearrange(
                "(k p) c -> p k c", p=128
            )
            with ExitStack() as sA:
                tps_pool = sA.enter_context(
                    tc.tile_pool(name="tpsA", bufs=4, space="PSUM")
                )
                acc_pool = sA.enter_context(
                    tc.tile_pool(name="accA", bufs=1, space="PSUM")
                )

                def transpose_hid(ht):
                    for s2 in range(2):
                        pt = tps_pool.tile([128, 128], BF16, tag="tps", name="tps")
                        nc.tensor.transpose(
                            pt[:], hid_sb[s2][:, ht * 128 : (ht + 1) * 128], ident[:]
                        )
                        nc.vector.tensor_copy(
                            hidT[ht][:, s2 * 128 : (s2 + 1) * 128], pt[:]
                        )

                kv_acc = [
                    [
                        acc_pool.tile(
                            [128, 288], F32, tag=f"kvacc{s2}{b}", name=f"kvacc{s2}{b}"
                        )
                        for b in range(2)
                    ]
                    for s2 in range(2)
                ]
                transpose_hid(0)
                transpose_hid(1)
                for g in range(HT_TILES // KVG):
                    t = kvw_pool.tile(
                        [128, KVG * CKV_R], BF16, tag="kvw", name=f"kvw{g}"
                    )
                    nc.sync.dma_start(
                        t[:].rearrange("p (k c) -> p k c", k=KVG),
                        wa_kv_view[:, g * KVG : (g + 1) * KVG],
                    )
                    for j in range(KVG):
                        kt = g * KVG + j
                        if kt + 2 < HT_TILES:
                            transpose_hid(kt + 2)
                        for s2 in range(2):
                            for b in range(2):
                                nc.tensor.matmul(
                                    kv_acc[s2][b][:],
                                    hidT[kt][:, s2 * 128 : (s2 + 1) * 128],
                                    t[:, j * CKV_R + b * 288 : j * CKV_R + (b + 1) * 288],
                                    start=(kt == 0),
                                    stop=(kt == HT_TILES - 1),
                                )
                for s2 in range(2):
                    for b in range(2):
                        nc.scalar.copy(
                            lat_sb[s2][:, Q_LORA + b * 288 : Q_LORA + (b + 1) * 288],
                            kv_acc[s2][b][:],
                        )

            # wkvb prefetch on the scalar ring (executes ~kv-copy time, well
            # before the kv up-projection needs it)
            for kt in range(CKV_TILES):
                nc.scalar.dma_start(
                    wkvb_sb[kt][:], wkvb_d[kt * 128 : (kt + 1) * 128, :]
                )

            def rms_diag(col0, ncols, stat_base):
                """1/rms of lat_sb[:, col0:col0+ncols] per row, folded into a
                per-s2 diagonal matrix applied by the latent transposes."""
                nch = (ncols + 511) // 512
                for s2 in range(2):
                    for ch in range(nch):
                        w = min(512, ncols - ch * 512)
                        src = lat_sb[s2][:, col0 + ch * 512 : col0 + ch * 512 + w]
                        nc.vector.scalar_tensor_tensor(
                            rms_scratch[:, 0:w],
                            src,
                            1.0,
                            src,
                            op0=mybir.AluOpType.mult,
                            op1=mybir.AluOpType.mult,
                            accum_out=stat[:, stat_base + 3 * s2 + ch
                                           : stat_base + 3 * s2 + ch + 1],
                        )
                    for ch in range(1, nch):
                        nc.vector.tensor_add(
                            stat[:, stat_base + 3 * s2 : stat_base + 3 * s2 + 1],
                            stat[:, stat_base + 3 * s2 : stat_base + 3 * s2 + 1],
                            stat[:, stat_base + 3 * s2 + ch
                                 : stat_base + 3 * s2 + ch + 1],
                        )
                for s2 in range(2):
                    nc.scalar.activation(
                        stat[:, stat_base + 6 + s2 : stat_base + 7 + s2],
                        stat[:, stat_base + 3 * s2 : stat_base + 3 * s2 + 1],
                        mybir.ActivationFunctionType.Sqrt,
                        scale=1.0 / ncols,
                        bias=eps_sb[:],
                    )
                for s2 in range(2):
                    nc.vector.reciprocal(
                        stat[:, stat_base + 8 + s2 : stat_base + 9 + s2],
                        stat[:, stat_base + 6 + s2 : stat_base + 7 + s2],
                    )
                for s2 in range(2):
                    nc.vector.tensor_scalar_mul(
                        diag_sb[s2][:],
                        ident[:],
                        stat[:, stat_base + 8 + s2 : stat_base + 9 + s2],
                    )

            def transpose_lat(src_col, w, dst, tps_pool, scaled):
                """dst[:, s2*128...] = (lat_sb[s2][:, src_col:src_col+w])T,
                optionally scaled per seq row (lat.T @ diag(1/rms))."""
                for s2 in range(2):
                    pt = tps_pool.tile([128, 128], F32, tag="tps", name="tpsl")
                    tmat = diag_sb[s2] if scaled else ident
                    nc.tensor.matmul(
                        pt[:w, :],
                        lat_sb[s2][:, src_col : src_col + w],
                        tmat[:],
                        start=True,
                        stop=True,
                    )
                    nc.vector.tensor_copy(
                        dst[:, s2 * 128 : (s2 + 1) * 128], pt[:w, :]
                    )

            with ExitStack() as sB:
                tpsB = sB.enter_context(tc.tile_pool(name="tpsB", bufs=2, space="PSUM"))
                rms_diag(Q_LORA, KV_LORA, 0)
                for ct in range(CKV_TILES):
                    transpose_lat(
                        Q_LORA + ct * 128,
                        128,
                        latTkv_all[:, ct * SL : (ct + 1) * SL],
                        tpsB,
                        scaled=True,
                    )
                transpose_lat(Q_LORA + KV_LORA, ROPE, latTkv_rope[:], tpsB, scaled=False)
                # latent stores on the scalar ring: the q weight stream on the
                # sync ring must not queue behind them
                nc.scalar.dma_start(
                    cc_in_kv[0:KV_LORA].rearrange("(ct p) s -> p ct s", p=128),
                    latTkv_all[:].rearrange("p (ct s) -> p ct s", ct=CKV_TILES),
                )
                nc.scalar.dma_start(
                    cc_in_kv[KV_LORA : KV_LORA + ROPE, :], latTkv_rope[:]
                )
            nc.gpsimd.collective_compute(
                "AllGather",
                mybir.AluOpType.bypass,
                replica_groups=[list(range(NCORES))],
                ins=[cc_in_kv[:].opt()],
                outs=[cc_out_kv[:].opt()],
            )

            # ---- q latents (three 512-col psum accumulators x 2 s2)
            with ExitStack() as sC:
                accC = sC.enter_context(tc.tile_pool(name="accC", bufs=1, space="PSUM"))
                q_acc = [
                    [
                        accC.tile(
                            [128, 512], F32, tag=f"qacc{s2}{b}", name=f"qacc{s2}{b}"
                        )
                        for b in range(3)
                    ]
                    for s2 in range(2)
                ]
                QG = 2  # kt per q weight DMA
                wa_q_view = wa_d[:, 0:Q_LORA].rearrange("(k p) c -> p k c", p=128)
                for g in range(HT_TILES // QG):
                    t = qw_pool.tile(
                        [128, QG * Q_LORA], BF16, tag="qw", name=f"qw{g}"
                    )
                    nc.sync.dma_start(
                        t[:].rearrange("p (k c) -> p k c", k=QG),
                        wa_q_view[:, g * QG : (g + 1) * QG],
                    )
                    for j in range(QG):
                        kt = g * QG + j
                        for s2 in range(2):
                            for b in range(3):
                                nc.tensor.matmul(
                                    q_acc[s2][b][:],
                                    hidT[kt][:, s2 * 128 : (s2 + 1) * 128],
                                    t[:, j * Q_LORA + b * 512 : j * Q_LORA + (b + 1) * 512],
                                    start=(kt == 0),
                                    stop=(kt == HT_TILES - 1),
                                )
                for s2 in range(2):
                    for b in range(3):
                        nc.scalar.copy(
                            lat_sb[s2][:, b * 512 : (b + 1) * 512], q_acc[s2][b][:]
                        )

            # wqb prefetch on the sync ring right behind the q weight stream
            for kt in range(CQ_TILES):
                nc.sync.dma_start(wqb_sb[kt][:], wqb_d[kt * 128 : (kt + 1) * 128, :])

            with ExitStack() as sD:
                tpsD = sD.enter_context(tc.tile_pool(name="tpsD", bufs=2, space="PSUM"))
                rms_diag(0, Q_LORA, 12)
                # chunk 0: ct 0-5 -> store -> gather; chunk 1: ct 6-11
                for ct in range(CQ_HALF):
                    transpose_lat(
                        ct * 128,
                        128,
                        latTq_all[:, ct * SL : (ct + 1) * SL],
                        tpsD,
                        scaled=True,
                    )
                nc.scalar.dma_start(
                    cc_in_q0[:].rearrange("(ct p) s -> p ct s", p=128),
                    latTq_all[:, 0 : CQ_HALF * SL].rearrange(
                        "p (ct s) -> p ct s", ct=CQ_HALF
                    ),
                )
                for ct in range(CQ_HALF, CQ_TILES):
                    transpose_lat(
                        ct * 128,
                        128,
                        latTq_all[:, ct * SL : (ct + 1) * SL],
                        tpsD,
                        scaled=True,
                    )
                nc.scalar.dma_start(
                    cc_in_q1[:].rearrange("(ct p) s -> p ct s", p=128),
                    latTq_all[:, CQ_HALF * SL :].rearrange(
                        "p (ct s) -> p ct s", ct=CQ_HALF
                    ),
                )
            nc.gpsimd.collective_compute(
                "AllGather",
                mybir.AluOpType.bypass,
                replica_groups=[list(range(NCORES))],
                ins=[cc_in_q0[:].opt()],
                outs=[cc_out_q0[:].opt()],
            )
            nc.gpsimd.collective_compute(
                "AllGather",
                mybir.AluOpType.bypass,
                replica_groups=[list(range(NCORES))],
                ins=[cc_in_q1[:].opt()],
                outs=[cc_out_q1[:].opt()],
            )

            # gathered kv latents into SBUF. The sync/scalar rings are idle
            # once phase 1 drains and (unlike gpsimd) not blocked by the
            # in-flight q collectives; each entry's gate time is later than
            # the previous entry's, so no head-of-line blocking.
            cc_kv_view = cc_out_kv[:].rearrange("j c s -> c j s")
            for i in range(CKV_TILES):
                eng = nc.sync if i < 2 else nc.scalar
                eng.dma_start(
                    latkv_sb[i][:].rearrange("c (j s) -> c j s", j=NCORES),
                    cc_kv_view[i * 128 : (i + 1) * 128],
                )
            for half in range(2):
                nc.scalar.dma_start(
                    kpe2[half * 64 : (half + 1) * 64, :].rearrange(
                        "c (j s) -> c j s", j=NCORES
                    ),
                    cc_kv_view[KV_LORA : KV_LORA + ROPE],
                )

        # ---------------- phase 2
        kvpool = stack.enter_context(tc.tile_pool(name="kvpool", bufs=1))
        knopeT = [
            kvpool.tile([128, S], BF16, tag=f"knopeT{h}", name=f"knopeT{h}")
            for h in range(HL)
        ]
        v_sb = [
            kvpool.tile([128, HL * V_DIM], BF16, tag=f"v{st}", name=f"v{st}")
            for st in range(S_TILES)
        ]
        qT = stack.enter_context(tc.tile_pool(name="qT", bufs=1))
        qTA = [qT.tile([128, S], BF16, tag=f"qTA{h}", name=f"qTA{h}") for h in range(HL)]
        # rope q of head pair (2p, 2p+1) stacked in partition halves
        qTB = [qT.tile([128, S], BF16, tag=f"qTB{p}", name=f"qTB{p}") for p in range(NPAIR)]
        outT_pool = stack.enter_context(tc.tile_pool(name="outT", bufs=1))
        outT = [
            outT_pool.tile([128, S], BF16, tag=f"outT{h}", name=f"outT{h}")
            for h in range(HL)
        ]

        # kv up-proj interleaved with the chunked q up-proj (one scope so the
        # PSUM pools coexist: pkv 2 banks + pq 6 banks = 8)
        with ExitStack() as p2q:
            latq = p2q.enter_context(tc.tile_pool(name="latq", bufs=1))
            latq_sb = [
                latq.tile([128, S], BF16, tag=f"latq{ct}", name=f"latq{ct}")
                for ct in range(CQ_TILES)
            ]
            cc_q_views = [
                cc_out_q0[:].rearrange("j c s -> c j s"),
                cc_out_q1[:].rearrange("j c s -> c j s"),
            ]
            # gpsimd is blocked until the q collectives complete, so chunk-0
            # loads ride the sync/scalar rings only; chunk-1 loads (gated on
            # the last gather, exactly when gpsimd unblocks) use all three
            for ct in range(CQ_TILES):
                half, cth = divmod(ct, CQ_HALF)
                if half == 0:
                    eng = (nc.sync, nc.scalar)[ct % 2]
                else:
                    eng = (nc.sync, nc.scalar, nc.gpsimd)[ct % 3]
                eng.dma_start(
                    latq_sb[ct][:].rearrange("c (j s) -> c j s", j=NCORES),
                    cc_q_views[half][cth * 128 : (cth + 1) * 128],
                )
            pkv_pool = p2q.enter_context(tc.tile_pool(name="pkv", bufs=2, space="PSUM"))
            pq_pool = p2q.enter_context(tc.tile_pool(name="pq", bufs=1, space="PSUM"))

            def knope_upproj():
                for h in range(HL):
                    for skb in range(NSQB):
                        pk = pkv_pool.tile([128, SQB], F32, tag="pkv", name="pk")
                        for kt in range(CKV_TILES):
                            nc.tensor.matmul(
                                pk[:],
                                wkvb_sb[kt][
                                    :, h * (NOPE + V_DIM) : h * (NOPE + V_DIM) + NOPE
                                ],
                                latkv_sb[kt][:, skb * SQB : (skb + 1) * SQB],
                                start=(kt == 0),
                                stop=(kt == CKV_TILES - 1),
                            )
                        nc.vector.tensor_copy(
                            knopeT[h][:, skb * SQB : (skb + 1) * SQB], pk[:]
                        )

            def v_upproj(st_range):
                for st in st_range:
                    pv = pkv_pool.tile([128, HL * V_DIM], F32, tag="pkv", name="pv")
                    for kt in range(CKV_TILES):
                        rhs = wkvb_sb[kt][:].rearrange(
                            "c (h d) -> c h d", h=HL
                        )[:, :, NOPE:]
                        nc.tensor.matmul(
                            pv[:],
                            latkv_sb[kt][:, st * 128 : (st + 1) * 128],
                            rhs,
                            start=(kt == 0),
                            stop=(kt == CKV_TILES - 1),
                        )
                    nc.vector.tensor_copy(v_sb[st][:], pv[:])

            def q_upproj_block(sqb, pqs, kts):
                # wqb cols are host-reordered: [nope h0..h3 | rope h0..h3]
                for kt in kts:
                    for h in range(HL):
                        nc.tensor.matmul(
                            pqs[h][:],
                            wqb_sb[kt][:, h * NOPE : (h + 1) * NOPE],
                            latq_sb[kt][:, sqb * SQB : (sqb + 1) * SQB],
                            start=(kt == 0),
                            stop=(kt == CQ_TILES - 1),
                        )
                    for p in range(NPAIR):
                        nc.tensor.matmul(
                            pqs[HL + p][:],
                            wqb_sb[kt][
                                :, HL * NOPE + p * 128 : HL * NOPE + (p + 1) * 128
                            ],
                            latq_sb[kt][:, sqb * SQB : (sqb + 1) * SQB],
                            start=(kt == 0),
                            stop=(kt == CQ_TILES - 1),
                        )

            def q_copies(sqb, pqs):
                for h in range(HL):
                    nc.scalar.copy(
                        qTA[h][:, sqb * SQB : (sqb + 1) * SQB], pqs[h][:]
                    )
                for p in range(NPAIR):
                    nc.scalar.copy(
                        qTB[p][:, sqb * SQB : (sqb + 1) * SQB], pqs[HL + p][:]
                    )

            def q_psums():
                return [
                    pq_pool.tile([128, SQB], F32, tag=f"pq{u}", name=f"pq{u}")
                    for u in range(HL + NPAIR)
                ]

            # emission order tuned to data arrival: kv work (gated on the kv
            # gather) fills the q-gather window; V's second half fills the
            # chunk-0 -> chunk-1 gap
            knope_upproj()
            v_upproj(range(0, 8))
            pqs0 = q_psums()
            q_upproj_block(0, pqs0, range(0, CQ_HALF))
            v_upproj(range(8, S_TILES))
            q_upproj_block(0, pqs0, range(CQ_HALF, CQ_TILES))
            q_copies(0, pqs0)
            for sqb in range(1, NSQB):
                pqs = q_psums()
                q_upproj_block(sqb, pqs, range(0, CQ_HALF))
                q_upproj_block(sqb, pqs, range(CQ_HALF, CQ_TILES))
                q_copies(sqb, pqs)

        # o_proj weights: loaded late (SBUF freed by the q latents)
        wo_pool = stack.enter_context(tc.tile_pool(name="wo", bufs=1))
        wo_sb = [
            wo_pool.tile([128, HID], BF16, tag=f"wo{h}", name=f"wo{h}")
            for h in range(HL)
        ]
        for h in range(HL):
            nc.sync.dma_start(wo_sb[h][:], wo_d[h * 128 : (h + 1) * 128, :])

        # ---------------- attention (causal, block-skipped) + interleaved o_proj
        # bq-outer so each 512-row sq block's outT completes early; its o_proj
        # block is emitted as soon as the last head's epilogue drains, filling
        # attention-pipeline bubbles and spreading the output DMA.
        # Rowsums: DVE pair-adds halve the exp tiles (bf16), then one
        # accumulating ones-matmul run per (h, bq) over the nk/2 pair-sums
        # (half the PE stream cost of per-tile rowsum matmuls); 1/rowsum is
        # exp(-ln(x)) on ACT and the renorm broadcast a bf16 K=1 matmul,
        # both deferred as in the baseline pipeline.
        with ExitStack() as p2a:
            ps_pool = p2a.enter_context(tc.tile_pool(name="ps", bufs=5, space="PSUM"))
            psum_sum_pool = p2a.enter_context(
                tc.tile_pool(name="psums", bufs=1, space="PSUM")
            )
            psum_o_pool = p2a.enter_context(
                tc.tile_pool(name="psumo", bufs=2, space="PSUM")
            )
            a_pool = p2a.enter_context(tc.tile_pool(name="apool", bufs=10))
            apair_pool = p2a.enter_context(tc.tile_pool(name="apair", bufs=6))
            aquad_pool = p2a.enter_context(tc.tile_pool(name="aquad", bufs=6))
            bc_pool = p2a.enter_context(tc.tile_pool(name="bcpool", bufs=3))

            tile_q = []  # score tiles awaiting their AV matmuls
            ep_q = []  # pairs awaiting the renormalization epilogue

            def drain_tile():
                a, h, bq, tk, nk, po = tile_q.pop(0)
                nc.tensor.matmul(
                    po[:],
                    v_sb[tk][:, h * V_DIM : (h + 1) * V_DIM],
                    a[:],
                    start=(tk == 0),
                    stop=(tk == nk - 1),
                )

            def drain_sums(pair):
                h, bq, po, psum, pair_sums = pair
                for i, ap in enumerate(pair_sums):
                    nc.tensor.matmul(
                        psum[:],
                        ones_sb[:],
                        ap[:],
                        start=(i == 0),
                        stop=(i == len(pair_sums) - 1),
                    )
                # 1/rowsum on DVE (custom op, ~18 bits, rowsum > 0 always):
                # keeps Ln/Exp off the ACT engine, whose activation-table set
                # would thrash against the softmax Exp (1.3us reload per swap)
                rs32 = bc_pool.tile([1, SQB], F32, tag="rs32", name="rs32")
                nc.vector.reciprocal_approx_fast(rs32[:], psum[:])
                rs = bc_pool.tile([1, SQB], BF16, tag="rs", name="rs")
                nc.vector.tensor_copy(rs[:], rs32[:])
                ep_q.append((h, bq, po, rs))

            def drain_epilogue():
                h, bq, po, rs = ep_q.pop(0)
                bc_ps = ps_pool.tile([128, SQB], F32, tag="ps", name="bc_ps")
                nc.tensor.matmul(bc_ps[:], onesr_sb[:], rs[:], start=True, stop=True)
                bc_sb = bc_pool.tile([128, SQB], F32, tag="bc", name="bc_sb")
                nc.scalar.copy(bc_sb[:], bc_ps[:])
                nc.vector.tensor_mul(
                    outT[h][:, bq * SQB : (bq + 1) * SQB], po[:], bc_sb[:]
                )

            prev_pair = None
            for bq in range(NSQB):
                nk = 4 * (bq + 1)
                for h in range(HL):
                    off = 64 * (h % 2)
                    qTBh = qTB[h // 2]
                    po = psum_o_pool.tile([128, SQB], F32, tag="psumo", name="po")
                    psum = psum_sum_pool.tile([1, SQB], F32, tag="psums", name="psum")
                    pair_sums = []
                    pend_a = None
                    pend_pair = None
                    for tk in range(nk):
                        ps = ps_pool.tile([128, SQB], F32, tag="ps", name="ps")
                        nc.tensor.matmul(
                            ps[:],
                            knopeT[h][:, tk * 128 : (tk + 1) * 128],
                            qTA[h][:, bq * SQB : (bq + 1) * SQB],
                            start=True,
                            stop=False,
                        )
                        nc.tensor.matmul(
                            ps[:],
                            kpe2[off : off + 64, tk * 128 : (tk + 1) * 128],
                            qTBh[off : off + 64, bq * SQB : (bq + 1) * SQB],
                            start=False,
                            stop=True,
                        )
                        d = tk - 4 * bq
                        if d >= 0:
                            nc.vector.tensor_add(
                                ps[:], ps[:], mask_sb[:, d * SQB : (d + 1) * SQB]
                            )
                        a = a_pool.tile([128, SQB], BF16, tag="a", name="a")
                        nc.scalar.activation(
                            a[:], ps[:], mybir.ActivationFunctionType.Exp
                        )
                        # rowsum pre-reduction: two DVE add levels (bf16) so
                        # each (h, bq) needs only nk/4 ones-matmul streams
                        if tk % 2 == 0:
                            pend_a = a
                        else:
                            apair = apair_pool.tile(
                                [128, SQB], BF16, tag="apair", name="apair"
                            )
                            nc.vector.tensor_add(apair[:], pend_a[:], a[:])
                            if tk % 4 == 1:
                                pend_pair = apair
                            else:
                                aquad = aquad_pool.tile(
                                    [128, SQB], BF16, tag="aquad", name="aquad"
                                )
                                nc.vector.tensor_add(
                                    aquad[:], pend_pair[:], apair[:]
                                )
                                pair_sums.append(aquad)
                        tile_q.append((a, h, bq, tk, nk, po))
                        while len(tile_q) > 3:
                            drain_tile()
                        if tk == 2 and prev_pair is not None:
                            drain_sums(prev_pair)
                            prev_pair = None
                        while len(ep_q) > 1:
                            drain_epilogue()
                    prev_pair = (h, bq, po, psum, pair_sums)
            while tile_q:
                drain_tile()
            if prev_pair is not None:
                drain_sums(prev_pair)
            while ep_q:
                drain_epilogue()

        # ---------------- o_proj (partial: summed across cores on host)
        # kept as a dedicated tail phase: its matmuls have trivially-satisfied
        # dependencies there and pipeline back-to-back on the PE
        with ExitStack() as p2o:
            pe_pool = p2o.enter_context(tc.tile_pool(name="pe", bufs=4, space="PSUM"))
            stage_pool = p2o.enter_context(tc.tile_pool(name="stage", bufs=3))
            for st in range(S_TILES):
                for half in range(2):
                    stg = stage_pool.tile([128, 4 * EB], BF16, tag="stage", name="stg")
                    for ebl in range(4):
                        eb = half * 4 + ebl
                        pe = pe_pool.tile([128, EB], F32, tag="pe", name="pe")
                        for h in range(HL):
                            nc.tensor.matmul(
                                pe[:],
                                outT[h][:, st * 128 : (st + 1) * 128],
                                wo_sb[h][:, eb * EB : (eb + 1) * EB],
                                start=(h == 0),
                                stop=(h == HL - 1),
                            )
                        nc.vector.tensor_copy(
                            stg[:, ebl * EB : (ebl + 1) * EB], pe[:]
                        )
                    nc.gpsimd.dma_start(
                        out_d[
                            st * 128 : (st + 1) * 128,
                            half * 4 * EB : (half + 1) * 4 * EB,
                        ],
                        stg[:],
                    )

    nc.compile()
    return nc


_PROGRAM_CACHE = {}


def _get_program() -> bass.Bass:
    if "nc" not in _PROGRAM_CACHE:
        _PROGRAM_CACHE["nc"] = build_program()
    return _PROGRAM_CACHE["nc"]


def prepare_inputs(
    hidden_states, w_qkv_a, q_a_gamma, w_q_b, kv_a_gamma, w_kv_b, w_o, b_o
):
    """Host-side prep: fold gammas + attention scale into B weights, cast to
    bf16, slice per core."""
    bf = ml_dtypes.bfloat16
    hs = np.asarray(hidden_states, np.float32).reshape(S, HID)
    scale = float(Q_HEAD) ** -0.5
    wqb_eff = (
        np.asarray(w_q_b, np.float32)
        * np.asarray(q_a_gamma, np.float32)[:, None]
        * scale
    )
    wkvb_eff = (
        np.asarray(w_kv_b, np.float32) * np.asarray(kv_a_gamma, np.float32)[:, None]
    )
    wa_bf = np.asarray(w_qkv_a, np.float32).astype(bf)
    hs_bf = hs.astype(bf)

    wqb_r = wqb_eff.reshape(Q_LORA, H, Q_HEAD)
    wkvb_r = wkvb_eff.reshape(KV_LORA, H, NOPE + V_DIM)
    wo_r = np.asarray(w_o, np.float32).reshape(H, V_DIM, HID)

    in_maps = []
    for c in range(NCORES):
        hsl = np.ascontiguousarray(hs_bf[c * SL : (c + 1) * SL])
        wqb_loc = wqb_r[:, c * HL : (c + 1) * HL]  # [Q_LORA, HL, Q_HEAD]
        # column order: [nope h0..h3 | rope h0..h3] (pair-packed rope)
        wqb_c = np.ascontiguousarray(
            np.concatenate(
                [
                    wqb_loc[:, :, :NOPE].reshape(Q_LORA, HL * NOPE),
                    wqb_loc[:, :, NOPE:].reshape(Q_LORA, HL * ROPE),
                ],
                axis=1,
            ).astype(bf)
        )
        wkvb_c = np.ascontiguousarray(
            wkvb_r[:, c * HL : (c + 1) * HL]
            .reshape(KV_LORA, HL * (NOPE + V_DIM))
            .astype(bf)
        )
        wo_c = np.ascontiguousarray(
            wo_r[c * HL : (c + 1) * HL].reshape(HL * V_DIM, HID).astype(bf)
        )
        in_maps.append(
            {
                "hid": hsl,
                "wa": wa_bf,
                "wqb": wqb_c,
                "wkvb": wkvb_c,
                "wo": wo_c,
            }
        )
    return in_maps


def kernel(**inputs) -> np.ndarray:
    in_maps = prepare_inputs(**inputs)
    nc = _get_program()
    res = run_bass_kernel_spmd(nc, in_maps, list(range(NCORES)))
    out = np.zeros((S, HID), np.float64)
    for r in res.results:
        out += np.asarray(r["out"], np.float32)
    out = out.astype(np.float32) + np.asarray(inputs["b_o"], np.float32)[None, :]
    return out.reshape(1, S, HID)
